# revision 49
# baseline (speedup 1.0000x reference)
"""BiLSTM-CRF (Viterbi decode) Trainium2 Bass kernel, 8-core data-parallel.

Full inputs in, full outputs out. Batch (64) is sharded 8 ways; each core runs:
  embedding gather -> input matmuls (gx = x @ Wih^T + b) -> 256-step fused
  fwd+bwd LSTM recurrence -> fc emissions -> Viterbi scan -> batched
  backpointer extraction -> backtrace.

Layout convention on device ("version B"): gate/hidden dims live on SBUF
partitions, batch on the free dim, so ACT/DVE use all 128 lanes.
"""

import os
import sys
import types

for _p in ('/opt/trn_rl_repo', '/root/.axon_site'):
    if _p not in sys.path:
        sys.path.insert(0, _p)

import numpy as np
import ml_dtypes

# ---- NTFF profile hook (lets run_bass_kernel_spmd(trace=True) return timings
# under axon; harmless if already registered or unavailable) ----
def _install_ntff_hook():
    try:
        import antenv
        if 'antenv.axon_hooks' in sys.modules:
            return
        from trn_agent_boot.trn_boot import _ntff_profile_via_ctypes
        m = types.ModuleType('antenv.axon_hooks')
        m._hook = _ntff_profile_via_ctypes('/opt/axon/libaxon_pjrt.so')
        m.get_axon_ntff_profile_hook = lambda: m._hook
        m.set_axon_ntff_profile_hook = lambda h: setattr(m, '_hook', h)
        sys.modules['antenv.axon_hooks'] = m
        antenv.axon_hooks = m
    except Exception:
        pass


_install_ntff_hook()

import concourse.bass as bass
import concourse.tile as tile
from concourse import bacc, mybir
from concourse.bass import IndirectOffsetOnAxis
from concourse.bass_utils import run_bass_kernel_spmd

F32 = mybir.dt.float32
BF16 = mybir.dt.bfloat16
F8 = mybir.dt.float8e4
I32 = mybir.dt.int32

# Problem dims (hardcoded per contract)
V, E, HS, T, B = 30000, 256, 512, 256, 64
H = HS // 2          # 256 per-direction hidden
G = 4 * H            # 1024 gate rows per direction
K = 10               # tags
NC_ = 8              # cores
BL = B // NC_        # 8 sequences per core
NBT = BL * T         # 2048 (b,t) columns per core
NSLOT = NBT // 128   # 16 gather slots

# Gate reorder: torch rows [i, f, g, o] -> device order [i, f, o, g]
# (sigmoid block = chunks 0..5, tanh block = chunks 6..7)
_PERM = np.concatenate([
    np.arange(0, 2 * H),          # i, f
    np.arange(3 * H, 4 * H),      # o
    np.arange(2 * H, 3 * H),      # g
])


def _bf(x):
    return np.ascontiguousarray(np.asarray(x, np.float32).astype(ml_dtypes.bfloat16))


def _f32(x):
    return np.ascontiguousarray(np.asarray(x, np.float32))


def _pack_w(wih, whh, bih, bhh):
    """Per direction: returns (w_ih[128, 2*8*128], w_hh[...], bias[128, 8]) in
    lhsT tile layout w[p, kc, mc, m] = W[perm[mc*128+m], kc*128+p].
    Scalings: h is stored as h/2 on device (so Whh gets *2), and g-gate
    rows (last H after perm) are pre-scaled by 2 so sigmoid gives
    tanh(g) = 2*sig(2g) - 1."""
    out = []
    for W, hscale in ((wih, 1.0), (whh, 2.0)):
        Wp = np.asarray(W, np.float32)[_PERM] * hscale  # [G, Kdim]
        Wp[3 * H:] *= 2.0
        Kd = Wp.shape[1]
        t = Wp.reshape(8, 128, Kd // 128, 128)          # [mc, m, kc, p]
        t = np.transpose(t, (3, 2, 0, 1))               # [p, kc, mc, m]
        out.append(t.reshape(128, -1))
    b = (np.asarray(bih, np.float32) + np.asarray(bhh, np.float32))[_PERM].copy()
    b[3 * H:] *= 2.0
    b = b.reshape(8, 128).T                             # [p, mc]
    return out[0], out[1], b


def _prep_core(inputs, core):
    """Host-side prep of all per-core device inputs."""
    s = slice(core * BL, (core + 1) * BL)
    inp = np.asarray(inputs['inp'])[s]        # [8, 256] int
    n = np.asarray(inputs['n'])[s].astype(np.int64)

    t_idx = np.arange(T)
    mask = t_idx[None, :] < n[:, None]
    rev = np.where(mask, n[:, None] - 1 - t_idx[None, :], t_idx[None, :])
    tok_rev = np.take_along_axis(inp, rev, axis=1)

    def idx_pack(tok):  # [8,256] -> [128, 16] slot layout (j = s*128+p, j=b*256+t)
        flat = np.asarray(tok, np.int64).reshape(-1)     # j = b*256+t
        return flat.reshape(NSLOT, 128).T.astype(np.int32).copy()

    wf = _pack_w(inputs['W_ih_f'], inputs['W_hh_f'], inputs['b_ih_f'], inputs['b_hh_f'])
    wb = _pack_w(inputs['W_ih_b'], inputs['W_hh_b'], inputs['b_ih_b'], inputs['b_hh_b'])
    w_ih = _bf(np.concatenate([wf[0], wb[0]], axis=1))   # [128, 2*2048]
    w_hh = _bf(np.concatenate([wf[1], wb[1]], axis=1))
    bias = _f32(np.concatenate([wf[2], wb[2]], axis=1))  # [128, 16] (d, mc)

    fcw = np.asarray(inputs['fc_w'], np.float32) * 2.0   # [10, 512]; h stored as h/2
    fcw_t = fcw.T.reshape(4, 128, K).transpose(1, 0, 2).reshape(128, 4 * K)
    fcbR = np.tile(np.asarray(inputs['fc_b'], np.float32)[None, :], (128, 1))

    trans = np.asarray(inputs['transition'], np.float32)[:K, :K]  # [prev, cur]
    transR = np.tile(trans.T.reshape(1, K * K), (128, 1)).copy()  # [p, cur*10+prev]

    iotaD = np.tile((9.0 - np.arange(K, dtype=np.float32))[None, :], (128, 1))
    iotaK = np.tile(np.arange(K, dtype=np.float32)[None, :], (128, 1))

    # validT[p, b*2+h] = ((h*128+p)+1 < n_b); ivT[p, (b,h,k)] = k*(1-valid)
    tt = (np.arange(256).reshape(2, 128).T)[:, None, :]            # [p, 1, h]
    validT = (tt + 1 < n[None, :, None]).astype(np.float32)        # [p, b, h]
    ivT = (1.0 - validT)[:, :, :, None] * np.arange(K, dtype=np.float32)[None, None, None, :]

    maskBT = np.zeros((128, T), np.float32)
    maskBT[:BL] = mask.astype(np.float32)

    # --- segmented viterbi/backtrace constants (partition p = b*16 + seg) ---
    transNAT = np.tile(trans.reshape(1, K * K), (128, 1)).copy()   # [p, k*10+j]
    idn = np.full((K, K), -1e9, np.float32)
    np.fill_diagonal(idn, 0.0)
    identneg = np.tile(idn.reshape(1, K * K), (128, 1)).copy()
    pb = np.arange(128) // 16                                       # seq of partition
    pseg = np.arange(128) % 16
    tgrid = pseg[:, None] * 16 + np.arange(16)[None, :]             # [p, tl] -> t
    mOB = (tgrid < n[pb][:, None]).astype(np.float32)               # emission valid
    mTR = mOB * (tgrid != 0)                                        # transition valid
    invA = 1.0 - mOB
    endrow = (np.arange(BL) * T + (n - 1)).astype(np.int32).reshape(BL, 1)

    # hb re-reversal gather rows: out col j=(b,t) <- hb_dram row b*256 + scan_idx
    scan_idx = np.where(mask, n[:, None] - 1 - t_idx[None, :], t_idx[None, :])
    hb_rows = ((scan_idx // 16) * 128 + (scan_idx % 16) * 8
               + np.arange(BL)[:, None]).reshape(-1)
    hb_off = hb_rows.reshape(NSLOT, 128).T.astype(np.int32).copy()

    return {
        'emb': _bf(inputs['emb']),
        'xidx': idx_pack(inp),
        'xridx': idx_pack(tok_rev),
        'w_ih': w_ih, 'w_hh': w_hh, 'bias32': bias,
        'fcw': _bf(fcw_t), 'fcbR': fcbR,
        'ident': np.eye(128, dtype=np.float32),
        'ident_bf': _bf(np.eye(128, dtype=np.float32)),
        'transR': transR, 'iotaD': iotaD, 'iotaK': iotaK,
        'validT': _f32(validT.reshape(128, 16)),
        'ivT': _f32(ivT.reshape(128, 160)),
        'maskBT': maskBT,
        'hb_off': hb_off,
        'transNAT': transNAT, 'identneg': identneg,
        'mTR': _f32(mTR), 'mOB': _f32(mOB), 'invA': _f32(invA),
        'endrow': endrow,
    }


# ----------------------------------------------------------------------------
# Device kernel
# ----------------------------------------------------------------------------

PHASE = int(os.environ.get('KPHASE', '9'))


def _build():
    nc = bacc.Bacc("TRN2", target_bir_lowering=False, debug=False,
                   num_devices=NC_)

    d_in = {}
    def din(name, shape, dt):
        d_in[name] = nc.dram_tensor(name, list(shape), dt, kind="ExternalInput").ap()
        return d_in[name]

    emb_d = din('emb', [V, E], BF16)
    xidx_d = din('xidx', [128, NSLOT], I32)
    xridx_d = din('xridx', [128, NSLOT], I32)
    wih_d = din('w_ih', [128, 2 * 2 * 8 * 128], BF16)
    whh_d = din('w_hh', [128, 2 * 2 * 8 * 128], BF16)
    bias_d = din('bias32', [128, 16], F32)
    fcw_d = din('fcw', [128, 4 * K], BF16)
    fcb_d = din('fcbR', [128, K], F32)
    id_d = din('ident', [128, 128], F32)
    idbf_d = din('ident_bf', [128, 128], BF16)
    trans_d = din('transR', [128, K * K], F32)
    iotaD_d = din('iotaD', [128, K], F32)
    iotaK_d = din('iotaK', [128, K], F32)
    validT_d = din('validT', [128, 16], F32)
    ivT_d = din('ivT', [128, 160], F32)
    maskBT_d = din('maskBT', [128, T], F32)
    hboff_d = din('hb_off', [128, NSLOT], I32)
    transNAT_d = din('transNAT', [128, K * K], F32)
    identneg_d = din('identneg', [128, K * K], F32)
    mTR_d = din('mTR', [128, 16], F32)
    mOB_d = din('mOB', [128, 16], F32)
    invA_d = din('invA', [128, 16], F32)
    endrow_d = din('endrow', [BL, 1], I32)

    out_d = nc.dram_tensor('out', [BL, T], F32, kind="ExternalOutput").ap()

    SIG = mybir.ActivationFunctionType.Sigmoid
    TANH = mybir.ActivationFunctionType.Tanh
    AL = mybir.AluOpType
    AX = mybir.AxisListType

    with tile.TileContext(nc) as tc:
        from contextlib import ExitStack
        ctx = ExitStack()
        cpool = ctx.enter_context(tc.tile_pool(name="consts", bufs=1))
        state = ctx.enter_context(tc.tile_pool(name="state", bufs=1))
        gather_p = ctx.enter_context(tc.tile_pool(name="gather", bufs=2))
        scratch = ctx.enter_context(tc.tile_pool(name="scratch", bufs=3))
        vit_p = ctx.enter_context(tc.tile_pool(name="vit", bufs=4))
        vbig = ctx.enter_context(tc.tile_pool(name="vbig", bufs=1))
        ps_tr = ctx.enter_context(tc.tile_pool(name="ps_tr", bufs=2, space="PSUM"))
        ps_mm = ctx.enter_context(tc.tile_pool(name="ps_mm", bufs=2, space="PSUM"))
        ps_g = ctx.enter_context(tc.tile_pool(name="ps_g", bufs=1, space="PSUM"))
        ps_fc = ctx.enter_context(tc.tile_pool(name="ps_fc", bufs=2, space="PSUM"))
        dram_p = ctx.enter_context(tc.tile_pool(name="dram", bufs=1, space="DRAM"))

        hb_dram_t = dram_p.tile([NBT, H], BF16)
        feats_dram_t = dram_p.tile([BL * T * K], F32)
        pre_dram_t = dram_p.tile([BL * T * K], F32)
        bp_dram_t = dram_p.tile([BL, T * K], F32)
        mz_dram_t = dram_p.tile([128 * K * K], F32)
        zb_dram_t = dram_p.tile([BL * 16 * K], F32)
        g_dram_t = dram_p.tile([128 * K], F32)
        r_dram_t = dram_p.tile([128], F32)
        b_dram_t = dram_p.tile([128 * 16], F32)
        hb_dram = hb_dram_t[:]
        feats_dram = feats_dram_t[:]
        pre_dram = pre_dram_t[:]
        bp_dram = bp_dram_t[:]
        mz_dram = mz_dram_t[:]
        zb_dram = zb_dram_t[:]
        g_dram = g_dram_t[:]
        r_dram = r_dram_t[:]
        b_dram = b_dram_t[:]

        def load_const(dram, shape, dt, tag):
            t = cpool.tile(shape, dt, tag=tag)
            nc.sync.dma_start(t[:], dram)
            return t

        wih = load_const(wih_d[:], [128, 4096], BF16, tag='wih')
        whh = load_const(whh_d[:], [128, 4096], BF16, tag='whh')
        bias = load_const(bias_d[:], [128, 16], F32, tag='bias')
        fcw = load_const(fcw_d[:], [128, 4 * K], BF16, tag='fcw')
        fcbR = load_const(fcb_d[:], [128, K], F32, tag='fcbR')
        ident = load_const(id_d[:], [128, 128], F32, tag='ident')
        ident_bf = load_const(idbf_d[:], [128, 128], BF16, tag='ident_bf')
        transR = load_const(trans_d[:], [128, K * K], F32, tag='transR')
        iotaD = load_const(iotaD_d[:], [128, K], F32, tag='iotaD')
        iotaK = load_const(iotaK_d[:], [128, K], F32, tag='iotaK')
        validT = load_const(validT_d[:], [128, 16], F32, tag='validT')
        ivT = load_const(ivT_d[:], [128, 160], F32, tag='ivT')
        maskBT = load_const(maskBT_d[:], [128, T], F32, tag='maskBT')
        xidx = load_const(xidx_d[:], [128, NSLOT], I32, tag='xidx')
        xridx = load_const(xridx_d[:], [128, NSLOT], I32, tag='xridx')
        hboff = load_const(hboff_d[:], [128, NSLOT], I32, tag='hboff')
        transNAT = load_const(transNAT_d[:], [128, K * K], F32, tag='transNAT')
        identneg = load_const(identneg_d[:], [128, K * K], F32, tag='identneg')
        mTR = load_const(mTR_d[:], [128, 16], F32, tag='mTR')
        mOB = load_const(mOB_d[:], [128, 16], F32, tag='mOB')
        invA = load_const(invA_d[:], [128, 16], F32, tag='invA')
        endrow = load_const(endrow_d[:], [BL, 1], I32, tag='endrow')

        wih_r = wih[:].rearrange("p (d kc mc m) -> p d kc mc m", d=2, kc=2, mc=8)
        whh_r = whh[:].rearrange("p (d kc mc m) -> p d kc mc m", d=2, kc=2, mc=8)
        fcw_r = fcw[:].rearrange("p (c k) -> p c k", c=4)

        # ---- P1: embedding gather + transpose to x^T (E on partitions) ----
        gx_ctx = ExitStack()
        gxpool = gx_ctx.enter_context(tc.tile_pool(name="gxp", bufs=1))
        x_ctx = ExitStack()
        xpool = x_ctx.enter_context(tc.tile_pool(name="xp", bufs=1))
        x_bf = xpool.tile([128, 2 * 2 * NBT], BF16)   # [p, dir, ec, bt]
        xbf_r = x_bf[:].rearrange("p (d e n) -> p d e n", d=2, e=2)
        for d, idxt in ((0, xidx), (1, xridx)):
            for s_ in range(NSLOT):
                xs = gather_p.tile([128, E], BF16, tag="xslot")
                nc.gpsimd.indirect_dma_start(
                    out=xs[:], out_offset=None, in_=emb_d,
                    in_offset=IndirectOffsetOnAxis(ap=idxt[:, s_:s_ + 1], axis=0),
                )
                for ec in range(2):
                    pt = ps_tr.tile([128, 128], BF16, tag="ptr")
                    nc.tensor.transpose(out=pt[:], in_=xs[:, ec * 128:(ec + 1) * 128],
                                        identity=ident_bf[:])
                    nc.vector.tensor_copy(
                        out=xbf_r[:, d, ec, s_ * 128:(s_ + 1) * 128], in_=pt[:])

        # ---- P2: gx = x @ Wih^T + bias (both dirs), bf16 store ----
        gx = gxpool.tile([128, 2 * 8 * NBT], BF16)     # [p, dir, mc, bt]
        gx_r = gx[:].rearrange("p (d mc n) -> p d mc n", d=2, mc=8)
        gx_rt = gx[:].rearrange("p (d mc b t) -> p d mc b t", d=2, mc=8, b=BL)
        NB = NBT // 512
        for d in range(2):
            for mc in range(8):
                for nb in range(NB):
                    pm = ps_mm.tile([128, 512], F32, tag="pmm")
                    for kc in range(2):
                        nc.tensor.matmul(
                            out=pm[:], lhsT=wih_r[:, d, kc, mc, :],
                            rhs=xbf_r[:, d, kc, nb * 512:(nb + 1) * 512],
                            start=(kc == 0), stop=(kc == 1))
                    nc.vector.tensor_scalar(
                        out=gx_r[:, d, mc, nb * 512:(nb + 1) * 512], in0=pm[:],
                        scalar1=bias[:, d * 8 + mc:d * 8 + mc + 1], scalar2=None,
                        op0=AL.add)

        x_ctx.close()

        # ---- P3: LSTM scan, two independent direction chains interleaved ----
        # Per-dir tiles so the chains share no state: PE does chain d's
        # matmuls while DVE/ACT run the other chain's nonlinearity.
        halls = []
        for d in range(2):
            h_d = state.tile([128, 2 * (T + 1) * BL], BF16,   # [p, kc, t, b]
                             tag=f"hall{d}", name=f"hall{d}")
            halls.append(h_d[:].rearrange("p (kc t b) -> p kc t b", kc=2, t=T + 1))
            nc.vector.memset(halls[d][:, :, 0, :], 0.0)
        cprev = [None, None]
        for d in range(2):
            c0 = scratch.tile([128, 2 * BL], F32, tag=f"c{d}")
            nc.vector.memset(c0[:], 0.0)
            cprev[d] = c0

        # per chain-step (state c2 = 2c, h stored as h/2):
        #   pg   = gx (identity matmul) + sum_kc Whh.h          [PSUM]
        #   sg   = sigmoid(pg)         (g-rows prescaled: sg_g = sig(2g))
        #   t1h  = (sg_g - 0.5) * sg_i                          [stt]
        #   cf2  = sg_f * c2_prev                               [tt]
        #   c2   = 4*t1h + cf2                                  [stt]
        #   sigc = sigmoid(c2)
        #   h/2  = (sigc - 0.5) * sg_o -> halls bf16            [stt]
        def emit_hb_slot(s_):
            # bwd-dir h slots [1+16s, 16+16s] -> transpose -> DRAM rows
            hbs = gather_p.tile([128, H], BF16, tag="hbs")
            for ec in range(2):
                pt = ps_tr.tile([128, 128], BF16, tag="ptr")
                nc.tensor.transpose(
                    out=pt[:],
                    in_=halls[1][:, ec, 1 + s_ * 16:1 + (s_ + 1) * 16, :],
                    identity=ident_bf[:])
                nc.vector.tensor_copy(out=hbs[:, ec * 128:(ec + 1) * 128],
                                      in_=pt[:])
            nc.sync.dma_start(out=hb_dram[s_ * 128:(s_ + 1) * 128, :], in_=hbs[:])

        pgs = {}

        def emit_idmm(t):
            # gx injection into PSUM; independent of h so it runs on the PE
            # while the previous step's nonlinearity is in flight
            for d in range(2):
                pg = ps_g.tile([128, 8 * BL], F32, tag=f"pg{d}")   # [p, mc, b]
                nc.tensor.matmul(out=pg[:], lhsT=ident_bf[:],
                                 rhs=gx_rt[:, d, :, :, t].rearrange(
                                     "p mc b -> p (mc b)"),
                                 start=True, stop=False)
                pgs[d] = pg

        emit_idmm(0)
        for t in range(T):
            sigs, t1s, cfs, sigcs = ({} for _ in range(4))
            curpg = dict(pgs)
            for d in range(2):
                pg_r = curpg[d][:].rearrange("p (mc b) -> p mc b", mc=8)
                for mc in range(8):
                    for kc in range(2):
                        nc.tensor.matmul(
                            out=pg_r[:, mc, :], lhsT=whh_r[:, d, kc, mc, :],
                            rhs=halls[d][:, kc, t, :],
                            start=False, stop=(mc == 7 and kc == 1))
            for d in range(2):
                sig = scratch.tile([128, 8 * BL], F32, tag=f"sig{d}")
                nc.scalar.activation(out=sig[:], in_=curpg[d][:], func=SIG)
                sigs[d] = sig[:].rearrange("p (c b) -> p c b", c=8)
            for d in range(2):
                t1 = scratch.tile([128, 2 * BL], F32, tag=f"t1{d}")
                nc.vector.scalar_tensor_tensor(
                    out=t1[:].rearrange("p (c b) -> p c b", c=2),
                    in0=sigs[d][:, 6:8, :], scalar=0.5, in1=sigs[d][:, 0:2, :],
                    op0=AL.subtract, op1=AL.mult)
                t1s[d] = t1
            for d in range(2):
                cf = scratch.tile([128, 2 * BL], F32, tag=f"cf{d}")
                nc.vector.tensor_mul(
                    out=cf[:].rearrange("p (c b) -> p c b", c=2),
                    in0=sigs[d][:, 2:4, :],
                    in1=cprev[d][:].rearrange("p (c b) -> p c b", c=2))
                cfs[d] = cf
            for d in range(2):
                cn = scratch.tile([128, 2 * BL], F32, tag=f"c{d}")
                nc.vector.scalar_tensor_tensor(
                    out=cn[:], in0=t1s[d][:], scalar=4.0, in1=cfs[d][:],
                    op0=AL.mult, op1=AL.add)
                cprev[d] = cn
            for d in range(2):
                sigc = scratch.tile([128, 2 * BL], F32, tag=f"sigc{d}")
                nc.scalar.activation(out=sigc[:], in_=cprev[d][:], func=SIG)
                sigcs[d] = sigc
            for d in range(2):
                nc.vector.scalar_tensor_tensor(
                    out=halls[d][:, :, t + 1, :],
                    in0=sigcs[d][:].rearrange("p (c b) -> p c b", c=2),
                    scalar=0.5, in1=sigs[d][:, 4:6, :],
                    op0=AL.subtract, op1=AL.mult)
            if t + 1 < T:
                emit_idmm(t + 1)

        gx_ctx.close()

        # ---- P4: hb re-reversal (DRAM bounce + indirect gather + transpose),
        #          then fc emissions ----
        for s_ in range(NSLOT):
            emit_hb_slot(s_)
        hbT = state.tile([128, 2 * NBT], BF16)   # [p(hid), kc, bt]
        hbT_r = hbT[:].rearrange("p (kc n) -> p kc n", kc=2)
        for s_ in range(NSLOT):
            hs = gather_p.tile([128, H], BF16, tag="hslot")
            nc.gpsimd.indirect_dma_start(
                out=hs[:], out_offset=None, in_=hb_dram,
                in_offset=IndirectOffsetOnAxis(ap=hboff[:, s_:s_ + 1], axis=0))
            for ec in range(2):
                pt = ps_tr.tile([128, 128], BF16, tag="ptr")
                nc.tensor.transpose(out=pt[:], in_=hs[:, ec * 128:(ec + 1) * 128],
                                    identity=ident_bf[:])
                nc.vector.tensor_copy(out=hbT_r[:, ec, s_ * 128:(s_ + 1) * 128], in_=pt[:])

        feats_sb = state.tile([128, 16 * K], F32)   # [p, mt, k], bt = mt*128+p
        feats_r = feats_sb[:].rearrange("p (m k) -> p m k", m=16)
        for mt in range(16):
            b_, th = mt // 2, mt % 2
            pf = ps_fc.tile([128, K], F32, tag="pfc")
            for c4 in range(4):
                if c4 < 2:
                    lhs = halls[0][:, c4, 1 + th * 128:1 + (th + 1) * 128, b_]
                else:
                    lhs = hbT_r[:, c4 - 2, mt * 128:(mt + 1) * 128]
                nc.tensor.matmul(out=pf[:], lhsT=lhs, rhs=fcw_r[:, c4, :],
                                 start=(c4 == 0), stop=(c4 == 3))
            nc.vector.tensor_tensor(out=feats_r[:, mt, :], in0=pf[:],
                                    in1=fcbR[:, :], op=AL.add)

        # relayout feats -> [(b,seg) partitions, (tl, k)] for segmented viterbi
        nc.sync.dma_start(
            out=feats_dram.rearrange("(b th p k) -> p b th k", b=BL, th=2, p=128),
            in_=feats_r[:, :, :].rearrange("p (b th) k -> p b th k", b=BL))
        # partition p = b*16 + seg (seg = th*8 + s2); t = seg*16 + tl
        f8seg = state.tile([128, 16 * K], F32)
        nc.sync.dma_start(
            out=f8seg[:],
            in_=feats_dram.rearrange("(b th s2 tl k) -> (b th s2) (tl k)",
                                     b=BL, th=2, s2=8, tl=16))
        f8_r = f8seg[:].rearrange("p (tl k) -> p tl k", tl=16)

        # ---- P5a: build A_t tiles, natural layout A[p, tl, k(prev), j(cur)] ----
        # A = mTR*trans + mOB*ob(bcast k) + invA*identneg
        NSEG = 16
        apool = ctx.enter_context(tc.tile_pool(name="apool", bufs=1))
        vs_p = ctx.enter_context(tc.tile_pool(name="vs", bufs=2))
        At = apool.tile([128, NSEG * K * K], F32)
        At_r = At[:].rearrange("p (tl k j) -> p tl k j", tl=NSEG, k=K)
        tmpA = vbig.tile([128, NSEG * K * K], F32, tag="tmpA")
        tmpA_r = tmpA[:].rearrange("p (tl k j) -> p tl k j", tl=NSEG, k=K)
        tmpO = vbig.tile([128, NSEG * K], F32, tag="tmpO")
        tmpO_r = tmpO[:].rearrange("p (tl j) -> p tl j", tl=NSEG)
        nc.vector.tensor_tensor(
            out=At_r[:, :, :, :],
            in0=transNAT[:, :].rearrange("p (k j) -> p k j", k=K).unsqueeze(1)
                .broadcast_to((128, NSEG, K, K)),
            in1=mTR[:, :].unsqueeze(2).unsqueeze(3).broadcast_to((128, NSEG, K, K)),
            op=AL.mult)
        nc.vector.tensor_tensor(
            out=tmpA_r[:, :, :, :],
            in0=identneg[:, :].rearrange("p (k j) -> p k j", k=K).unsqueeze(1)
                .broadcast_to((128, NSEG, K, K)),
            in1=invA[:, :].unsqueeze(2).unsqueeze(3).broadcast_to((128, NSEG, K, K)),
            op=AL.mult)
        nc.vector.tensor_add(out=At[:], in0=At[:], in1=tmpA[:])
        nc.vector.tensor_tensor(
            out=tmpO_r[:, :, :], in0=f8_r[:, :, :],
            in1=mOB[:, :].unsqueeze(2).broadcast_to((128, NSEG, K)), op=AL.mult)
        nc.vector.tensor_tensor(
            out=At_r[:, :, :, :], in0=At_r[:, :, :, :],
            in1=tmpO_r[:, :, :].unsqueeze(2).broadcast_to((128, NSEG, K, K)),
            op=AL.add)

        # ---- P5b: phase 1 — per-segment max-plus matrix composition ----
        # M[p, i, k] ; step: M'[i, j] = max_k(M[i, k] + A[tl][k, j])
        Mt = vs_p.tile([128, K * K], F32, tag="Mt")
        nc.vector.tensor_copy(out=Mt[:], in_=At_r[:, 0, :, :])
        for tl in range(1, NSEG):
            sb = vs_p.tile([128, K * K * K], F32, tag="sb")
            sb_r = sb[:].rearrange("p (i j k) -> p i j k", i=K, j=K)
            nc.vector.tensor_tensor(
                out=sb_r[:, :, :, :],
                in0=Mt[:].rearrange("p (i k) -> p i k", i=K).unsqueeze(2)
                    .broadcast_to((128, K, K, K)),
                in1=At_r[:, tl, :, :].rearrange("p k j -> p j k").unsqueeze(1)
                    .broadcast_to((128, K, K, K)),
                op=AL.add)
            Mt = vs_p.tile([128, K * K], F32, tag="Mt")
            nc.vector.tensor_reduce(
                out=Mt[:].rearrange("p (i j) -> p i j", i=K),
                in_=sb_r[:, :, :, :], axis=AX.X, op=AL.max)

        # ---- P5c: boundary pass on 8 partitions: z_s = M_s (x) z_{s-1} ----
        nc.sync.dma_start(
            out=mz_dram.rearrange("(b s ij) -> (b s) ij", b=BL, s=NSEG),
            in_=Mt[:])
        M8 = vbig.tile([128, NSEG * K * K], F32, tag="M8")
        nc.sync.dma_start(out=M8[0:BL, :],
                          in_=mz_dram.rearrange("(b sij) -> b sij", b=BL))
        M8_r = M8[:].rearrange("p (s k j) -> p s k j", s=NSEG, k=K)
        zbuf = vbig.tile([128, NSEG * K], F32, tag="zbuf")
        zbuf_r = zbuf[:].rearrange("p (s k) -> p s k", s=NSEG)
        nc.vector.memset(zbuf[0:BL, :], 0.0)
        for s in range(NSEG - 1):
            s3 = vit_p.tile([128, K * K], F32, tag="s3")
            s3_r = s3[:].rearrange("p (j i) -> p j i", j=K)
            nc.vector.tensor_tensor(
                out=s3_r[0:BL, :, :],
                in0=zbuf_r[0:BL, s, :].unsqueeze(1).broadcast_to((BL, K, K)),
                in1=M8_r[0:BL, s, :, :].rearrange("p k j -> p j k"),
                op=AL.add)
            nc.vector.tensor_reduce(
                out=zbuf_r[0:BL, s + 1, :], in_=s3_r[0:BL, :, :],
                axis=AX.X, op=AL.max)

        # relayout z starts -> [(b,seg) partitions, k]
        nc.sync.dma_start(
            out=zb_dram.rearrange("(b sk) -> b sk", b=BL),
            in_=zbuf[0:BL, :])
        zstart = vs_p.tile([128, K], F32, tag="zstart")
        nc.sync.dma_start(out=zstart[:],
                          in_=zb_dram.rearrange("(p k) -> p k", p=128))

        # ---- P5d: phase 2 — within-segment forward scan, all segs parallel ----
        pre_seg = state.tile([128, NSEG * K], F32)
        pre_r = pre_seg[:].rearrange("p (tl k) -> p tl k", tl=NSEG)
        prev_ap = zstart[:, :]
        for tl in range(NSEG):
            s4 = vit_p.tile([128, K * K], F32, tag="s4")
            s4_r = s4[:].rearrange("p (j k) -> p j k", j=K)
            nc.vector.tensor_tensor(
                out=s4_r[:, :, :],
                in0=prev_ap.unsqueeze(1).broadcast_to((128, K, K)),
                in1=At_r[:, tl, :, :].rearrange("p k j -> p j k"),
                op=AL.add)
            nc.vector.tensor_reduce(
                out=pre_r[:, tl, :], in_=s4_r[:, :, :], axis=AX.X, op=AL.max)
            prev_ap = pre_r[:, tl, :]

        # dump pre -> pre_dram in (b t k) order
        nc.sync.dma_start(
            out=pre_dram.rearrange("(b th s2 tl k) -> (b th s2) (tl k)",
                                   b=BL, th=2, s2=8, tl=16),
            in_=pre_seg[:])

        # ---- P5e: end tag via indirect gather of pre[b, n_b-1, :] ----
        peG = vit_p.tile([128, K], F32, tag="peG")
        nc.gpsimd.indirect_dma_start(
            out=peG[0:BL, :], out_offset=None,
            in_=pre_dram.rearrange("(r k) -> r k", k=K),
            in_offset=IndirectOffsetOnAxis(ap=endrow[:, 0:1], axis=0))
        mvE = vit_p.tile([128, 1], F32, tag="mvE")
        nc.vector.tensor_reduce(out=mvE[0:BL, :], in_=peG[0:BL, :], axis=AX.X, op=AL.max)
        eqE = vit_p.tile([128, K], F32, tag="eqE")
        nc.vector.tensor_tensor(out=eqE[0:BL, :], in0=peG[0:BL, :],
                                in1=mvE[0:BL, :].broadcast_to((BL, K)), op=AL.is_equal)
        nc.vector.tensor_mul(out=eqE[0:BL, :], in0=eqE[0:BL, :], in1=iotaD[0:BL, :])
        endt8 = vit_p.tile([128, 1], F32, tag="endt8")
        nc.vector.tensor_reduce(out=endt8[0:BL, :], in_=eqE[0:BL, :], axis=AX.X, op=AL.max)
        nc.vector.tensor_scalar(out=endt8[0:BL, :], in0=endt8[0:BL, :],
                                scalar1=-1.0, scalar2=9.0, op0=AL.mult, op1=AL.add)

        # ---- P6: batched backpointer extraction (from pre_dram, (b t k)) ----
        preT = vbig.tile([128, 2 * BL * K], F32, tag="preT")
        nc.sync.dma_start(
            out=preT[:].rearrange("p (bh k) -> p bh k", bh=2 * BL),
            in_=pre_dram.rearrange("(b h p k) -> p (b h) k", b=BL, h=2, p=128))

        HB = 2 * BL
        preT_hb = preT[:].rearrange("p (hb k) -> p hb k", k=K)
        sX = vbig.tile([128, 2 * BL * K * K], F32, tag="sX")
        sX_r = sX[:].rearrange("p (hb c q) -> p hb c q", hb=HB, c=K)
        nc.vector.tensor_tensor(
            out=sX_r[:, :, :, :],
            in0=preT_hb.unsqueeze(2).broadcast_to((128, HB, K, K)),
            in1=transR[:, :].rearrange("p (c q) -> p c q", c=K).unsqueeze(1)
                .broadcast_to((128, HB, K, K)),
            op=AL.add)
        mX = vbig.tile([128, 2 * BL * K], F32, tag="mX")
        mX_r = mX[:].rearrange("p (hb c) -> p hb c", hb=HB)
        nc.vector.tensor_reduce(out=mX_r[:, :, :], in_=sX_r[:, :, :, :],
                                axis=AX.X, op=AL.max)
        eq = vbig.tile([128, 2 * BL * K * K], F32, tag="eq")
        eq_r = eq[:].rearrange("p (hb c q) -> p hb c q", hb=HB, c=K)
        nc.vector.tensor_tensor(
            out=eq_r[:, :, :, :], in0=sX_r[:, :, :, :],
            in1=mX_r[:, :, :].unsqueeze(3).broadcast_to((128, HB, K, K)),
            op=AL.is_equal)
        nc.vector.tensor_tensor(
            out=eq_r[:, :, :, :], in0=eq_r[:, :, :, :],
            in1=iotaD[:, :].unsqueeze(1).unsqueeze(1).broadcast_to((128, HB, K, K)),
            op=AL.mult)
        bq = vbig.tile([128, 2 * BL * K], F32, tag="bq")
        bq_r = bq[:].rearrange("p (hb c) -> p hb c", hb=HB)
        nc.vector.tensor_reduce(out=bq_r[:, :, :], in_=eq_r[:, :, :, :],
                                axis=AX.X, op=AL.max)
        # bp = 9 - bq ; then pad override: bp*valid + iota_cur*(1-valid)
        nc.vector.tensor_scalar(out=bq[:], in0=bq[:], scalar1=-1.0, scalar2=9.0,
                                op0=AL.mult, op1=AL.add)
        nc.vector.tensor_tensor(
            out=bq_r[:, :, :], in0=bq_r[:, :, :],
            in1=validT[:, :].unsqueeze(2).broadcast_to((128, HB, K)),
            op=AL.mult)
        nc.vector.tensor_tensor(
            out=bq_r[:, :, :], in0=bq_r[:, :, :],
            in1=ivT[:, :].rearrange("p (hb k) -> p hb k", k=K),
            op=AL.add)
        # bp_dram slot t holds the map f_{t+1} (transition into t+1)
        nc.sync.dma_start(
            out=bp_dram[:, :].rearrange("b (h p k) -> p (b h) k", h=2, p=128),
            in_=bq[:].rearrange("p (bh k) -> p bh k", bh=2 * BL))

        # ---- P7: segmented backtrace ----
        # ftile[p=(b,seg), tl, j] = f at u = seg*16+tl, for tl = 1..15
        # (slot u lives at bp_dram position u-1 = seg*16 + (tl-1))
        ftile = state.tile([128, NSEG * K], F32)
        nc.sync.dma_start(
            out=ftile[:, K:],
            in_=bp_dram[:, :].rearrange(
                "b (th s2 tl k) -> (b th s2) (tl k)", th=2, s2=8, tl=16)[:, 0:150])
        ft_r = ftile[:].rearrange("p (tl j) -> p tl j", tl=NSEG)
        # fend8[b, s-1, j] = f_{16s} (= bp_dram position 16s-1), s = 1..15
        fend8 = vbig.tile([128, 15 * K], F32, tag="fend8")
        nc.sync.dma_start(
            out=fend8[0:BL, :].rearrange("p (s k) -> p s k", s=15),
            in_=bp_dram[:, 150:2550].rearrange("b (s gk) -> b s gk",
                                               s=15)[:, :, 0:K])
        fend8_r = fend8[:].rearrange("p (s k) -> p s k", s=15)

        # phase 1: compose G'_s = f_{16s+1} o ... o f_{16s+15}
        Ct = vs_p.tile([128, K], F32, tag="Ct")
        nc.vector.tensor_copy(out=Ct[:], in_=ft_r[:, NSEG - 1, :])
        for tl in range(NSEG - 2, 0, -1):
            ohB = vit_p.tile([128, K * K], F32, tag="ohB")
            ohB_r = ohB[:].rearrange("p (i j) -> p i j", i=K)
            nc.vector.tensor_tensor(
                out=ohB_r[:, :, :],
                in0=Ct[:].unsqueeze(2).broadcast_to((128, K, K)),
                in1=iotaK[:, :].unsqueeze(1).broadcast_to((128, K, K)),
                op=AL.is_equal)
            nc.vector.tensor_tensor(
                out=ohB_r[:, :, :], in0=ohB_r[:, :, :],
                in1=ft_r[:, tl, :].unsqueeze(1).broadcast_to((128, K, K)),
                op=AL.mult)
            Ct = vs_p.tile([128, K], F32, tag="Ct")
            nc.vector.tensor_reduce(out=Ct[:], in_=ohB_r[:, :, :],
                                    axis=AX.X, op=AL.max)

        # relayout G -> [8, (s, i)]
        nc.sync.dma_start(
            out=g_dram.rearrange("(b s i) -> (b s) i", b=BL, s=NSEG), in_=Ct[:])
        G8 = vbig.tile([128, NSEG * K], F32, tag="G8")
        nc.sync.dma_start(out=G8[0:BL, :],
                          in_=g_dram.rearrange("(b si) -> b si", b=BL))
        G8_r = G8[:].rearrange("p (s i) -> p s i", s=NSEG)

        # boundary pass: r_{s-1} = f_{16s}(G'_s(r_s)), r_15 = end tag
        rbuf = vbig.tile([128, NSEG], F32, tag="rbuf")
        nc.vector.tensor_copy(out=rbuf[0:BL, NSEG - 1:NSEG], in_=endt8[0:BL, :])
        for s in range(NSEG - 1, 0, -1):
            oh8 = vit_p.tile([128, K], F32, tag="oh8")
            nc.vector.tensor_tensor(
                out=oh8[0:BL, :], in0=iotaK[0:BL, :],
                in1=rbuf[0:BL, s:s + 1].broadcast_to((BL, K)), op=AL.is_equal)
            nc.vector.tensor_mul(out=oh8[0:BL, :], in0=oh8[0:BL, :],
                                 in1=G8_r[0:BL, s, :])
            tG = vit_p.tile([128, 1], F32, tag="tG")
            nc.vector.tensor_reduce(out=tG[0:BL, :], in_=oh8[0:BL, :],
                                    axis=AX.X, op=AL.max)
            oh9 = vit_p.tile([128, K], F32, tag="oh9")
            nc.vector.tensor_tensor(
                out=oh9[0:BL, :], in0=iotaK[0:BL, :],
                in1=tG[0:BL, :].broadcast_to((BL, K)), op=AL.is_equal)
            nc.vector.tensor_mul(out=oh9[0:BL, :], in0=oh9[0:BL, :],
                                 in1=fend8_r[0:BL, s - 1, :])
            nc.vector.tensor_reduce(out=rbuf[0:BL, s - 1:s], in_=oh9[0:BL, :],
                                    axis=AX.X, op=AL.max)

        # relayout r -> [(b,seg) partitions, 1]
        nc.sync.dma_start(out=r_dram.rearrange("(b s) -> b s", b=BL),
                          in_=rbuf[0:BL, :])
        rstart = vs_p.tile([128, 1], F32, tag="rstart")
        nc.sync.dma_start(out=rstart[:],
                          in_=r_dram.rearrange("(p one) -> p one", one=1))

        # phase 2: walk back within each segment, all segs parallel
        bestseg = state.tile([128, NSEG], F32)
        nc.vector.tensor_copy(out=bestseg[:, NSEG - 1:NSEG], in_=rstart[:])
        for tl in range(NSEG - 1, 0, -1):
            oh2 = vit_p.tile([128, K], F32, tag="oh2")
            nc.vector.tensor_tensor(
                out=oh2[:, :], in0=iotaK[:, :],
                in1=bestseg[:, tl:tl + 1].broadcast_to((128, K)), op=AL.is_equal)
            nc.vector.tensor_mul(out=oh2[:, :], in0=oh2[:, :], in1=ft_r[:, tl, :])
            nc.vector.tensor_reduce(out=bestseg[:, tl - 1:tl], in_=oh2[:, :],
                                    axis=AX.X, op=AL.max)

        # bestseg[p=(b,seg), tl] -> [8, 256], mask, out
        nc.sync.dma_start(
            out=b_dram.rearrange("(b th s2 tl) -> (b th s2) tl",
                                 b=BL, th=2, s2=8),
            in_=bestseg[:])
        best8 = state.tile([128, T], F32)
        nc.sync.dma_start(out=best8[0:BL, :],
                          in_=b_dram.rearrange("(b t) -> b t", b=BL))
        bestM = state.tile([128, T], F32)
        nc.vector.tensor_mul(out=bestM[0:BL, :], in0=best8[0:BL, :],
                             in1=maskBT[0:BL, :])
        nc.sync.dma_start(out=out_d, in_=bestM[0:BL, :])
        ctx.close()

    nc.compile()
    return nc


_NC_CACHE = None


def _get_nc():
    global _NC_CACHE
    if _NC_CACHE is None:
        _NC_CACHE = _build()
    return _NC_CACHE


TRACE = False
LAST_EXEC_NS = None


def kernel(**inputs) -> np.ndarray:
    global LAST_EXEC_NS
    nc = _get_nc()
    in_maps = [_prep_core(inputs, c) for c in range(NC_)]
    res = run_bass_kernel_spmd(nc, in_maps, list(range(NC_)), trace=TRACE)
    LAST_EXEC_NS = res.exec_time_ns
    out = np.concatenate([res.results[c]['out'] for c in range(NC_)], axis=0)
    return out.astype(np.float32)


if __name__ == '__main__':
    _build()
    print("build ok")



# revision 53
# speedup vs baseline: 1.0458x; 1.0458x over previous
"""BiLSTM-CRF (Viterbi decode) Trainium2 Bass kernel, 8-core data-parallel.

Full inputs in, full outputs out. Batch (64) is sharded 8 ways; each core runs:
  embedding gather -> input matmuls (gx = x @ Wih^T + b) -> 256-step fused
  fwd+bwd LSTM recurrence -> fc emissions -> Viterbi scan -> batched
  backpointer extraction -> backtrace.

Layout convention on device ("version B"): gate/hidden dims live on SBUF
partitions, batch on the free dim, so ACT/DVE use all 128 lanes.
"""

import os
import sys
import types

for _p in ('/opt/trn_rl_repo', '/root/.axon_site'):
    if _p not in sys.path:
        sys.path.insert(0, _p)

import numpy as np
import ml_dtypes

# ---- NTFF profile hook (lets run_bass_kernel_spmd(trace=True) return timings
# under axon; harmless if already registered or unavailable) ----
def _install_ntff_hook():
    try:
        import antenv
        if 'antenv.axon_hooks' in sys.modules:
            return
        from trn_agent_boot.trn_boot import _ntff_profile_via_ctypes
        m = types.ModuleType('antenv.axon_hooks')
        m._hook = _ntff_profile_via_ctypes('/opt/axon/libaxon_pjrt.so')
        m.get_axon_ntff_profile_hook = lambda: m._hook
        m.set_axon_ntff_profile_hook = lambda h: setattr(m, '_hook', h)
        sys.modules['antenv.axon_hooks'] = m
        antenv.axon_hooks = m
    except Exception:
        pass


_install_ntff_hook()

import concourse.bass as bass
import concourse.tile as tile
from concourse import bacc, mybir
from concourse.bass import IndirectOffsetOnAxis
from concourse.bass_utils import run_bass_kernel_spmd

F32 = mybir.dt.float32
BF16 = mybir.dt.bfloat16
F8 = mybir.dt.float8e4
I32 = mybir.dt.int32

# Problem dims (hardcoded per contract)
V, E, HS, T, B = 30000, 256, 512, 256, 64
H = HS // 2          # 256 per-direction hidden
G = 4 * H            # 1024 gate rows per direction
K = 10               # tags
NC_ = 8              # cores
BL = B // NC_        # 8 sequences per core
NBT = BL * T         # 2048 (b,t) columns per core
NSLOT = NBT // 128   # 16 gather slots

# Gate reorder: torch rows [i, f, g, o] -> device order [i, f, o, g]
# (sigmoid block = chunks 0..5, tanh block = chunks 6..7)
_PERM = np.concatenate([
    np.arange(0, 2 * H),          # i, f
    np.arange(3 * H, 4 * H),      # o
    np.arange(2 * H, 3 * H),      # g
])


def _bf(x):
    return np.ascontiguousarray(np.asarray(x, np.float32).astype(ml_dtypes.bfloat16))


def _f32(x):
    return np.ascontiguousarray(np.asarray(x, np.float32))


def _pack_w(wih, whh, bih, bhh):
    """Per direction: returns (w_ih[128, 2*8*128], w_hh[...], bias[128, 8]) in
    lhsT tile layout w[p, kc, mc, m] = W[perm[mc*128+m], kc*128+p].
    Scalings: h is stored as h/2 on device (so Whh gets *2), and g-gate
    rows (last H after perm) are pre-scaled by 2 so sigmoid gives
    tanh(g) = 2*sig(2g) - 1."""
    out = []
    for W, hscale in ((wih, 1.0), (whh, 2.0)):
        Wp = np.asarray(W, np.float32)[_PERM] * hscale  # [G, Kdim]
        Wp[3 * H:] *= 2.0
        Kd = Wp.shape[1]
        t = Wp.reshape(8, 128, Kd // 128, 128)          # [mc, m, kc, p]
        t = np.transpose(t, (3, 2, 0, 1))               # [p, kc, mc, m]
        out.append(t.reshape(128, -1))
    b = (np.asarray(bih, np.float32) + np.asarray(bhh, np.float32))[_PERM].copy()
    b[3 * H:] *= 2.0
    b = b.reshape(8, 128).T                             # [p, mc]
    return out[0], out[1], b


def _prep_core(inputs, core):
    """Host-side prep of all per-core device inputs."""
    s = slice(core * BL, (core + 1) * BL)
    inp = np.asarray(inputs['inp'])[s]        # [8, 256] int
    n = np.asarray(inputs['n'])[s].astype(np.int64)

    t_idx = np.arange(T)
    mask = t_idx[None, :] < n[:, None]
    rev = np.where(mask, n[:, None] - 1 - t_idx[None, :], t_idx[None, :])
    tok_rev = np.take_along_axis(inp, rev, axis=1)

    def idx_pack(tok):  # [8,256] -> [128, 16] slot layout (j = s*128+p, j=b*256+t)
        flat = np.asarray(tok, np.int64).reshape(-1)     # j = b*256+t
        return flat.reshape(NSLOT, 128).T.astype(np.int32).copy()

    wf = _pack_w(inputs['W_ih_f'], inputs['W_hh_f'], inputs['b_ih_f'], inputs['b_hh_f'])
    wb = _pack_w(inputs['W_ih_b'], inputs['W_hh_b'], inputs['b_ih_b'], inputs['b_hh_b'])
    w_ih = _bf(np.concatenate([wf[0], wb[0]], axis=1))   # [128, 2*2048]
    w_hh = _bf(np.concatenate([wf[1], wb[1]], axis=1))
    bias = _f32(np.concatenate([wf[2], wb[2]], axis=1))  # [128, 16] (d, mc)

    fcw = np.asarray(inputs['fc_w'], np.float32) * 2.0   # [10, 512]; h stored as h/2
    fcw_t = fcw.T.reshape(4, 128, K).transpose(1, 0, 2).reshape(128, 4 * K)
    fcbR = np.tile(np.asarray(inputs['fc_b'], np.float32)[None, :], (128, 1))

    trans = np.asarray(inputs['transition'], np.float32)[:K, :K]  # [prev, cur]
    transR = np.tile(trans.T.reshape(1, K * K), (128, 1)).copy()  # [p, cur*10+prev]

    iotaD = np.tile((9.0 - np.arange(K, dtype=np.float32))[None, :], (128, 1))
    iotaK = np.tile(np.arange(K, dtype=np.float32)[None, :], (128, 1))

    # validT[p, b*2+h] = ((h*128+p)+1 < n_b); ivT[p, (b,h,k)] = k*(1-valid)
    tt = (np.arange(256).reshape(2, 128).T)[:, None, :]            # [p, 1, h]
    validT = (tt + 1 < n[None, :, None]).astype(np.float32)        # [p, b, h]
    ivT = (1.0 - validT)[:, :, :, None] * np.arange(K, dtype=np.float32)[None, None, None, :]

    maskBT = np.zeros((128, T), np.float32)
    maskBT[:BL] = mask.astype(np.float32)

    # --- segmented viterbi/backtrace constants (partition p = b*16 + seg) ---
    transNAT = np.tile(trans.reshape(1, K * K), (128, 1)).copy()   # [p, k*10+j]
    idn = np.full((K, K), -1e9, np.float32)
    np.fill_diagonal(idn, 0.0)
    identneg = np.tile(idn.reshape(1, K * K), (128, 1)).copy()
    pb = np.arange(128) // 16                                       # seq of partition
    pseg = np.arange(128) % 16
    tgrid = pseg[:, None] * 16 + np.arange(16)[None, :]             # [p, tl] -> t
    mOB = (tgrid < n[pb][:, None]).astype(np.float32)               # emission valid
    mTR = mOB * (tgrid != 0)                                        # transition valid
    invA = 1.0 - mOB
    endrow = (np.arange(BL) * T + (n - 1)).astype(np.int32).reshape(BL, 1)

    # hb re-reversal gather rows: out col j=(b,t) <- hb_dram row b*256 + scan_idx
    scan_idx = np.where(mask, n[:, None] - 1 - t_idx[None, :], t_idx[None, :])
    hb_rows = ((scan_idx // 16) * 128 + (scan_idx % 16) * 8
               + np.arange(BL)[:, None]).reshape(-1)
    hb_off = hb_rows.reshape(NSLOT, 128).T.astype(np.int32).copy()

    return {
        'emb': _bf(inputs['emb']),
        'xidx': idx_pack(inp),
        'xridx': idx_pack(tok_rev),
        'w_ih': w_ih, 'w_hh': w_hh, 'bias32': bias,
        'fcw': _bf(fcw_t), 'fcbR': fcbR,
        'ident': np.eye(128, dtype=np.float32),
        'ident_bf': _bf(np.eye(128, dtype=np.float32)),
        'transR': transR, 'iotaD': iotaD, 'iotaK': iotaK,
        'validT': _f32(validT.reshape(128, 16)),
        'ivT': _f32(ivT.reshape(128, 160)),
        'maskBT': maskBT,
        'hb_off': hb_off,
        'transNAT': transNAT, 'identneg': identneg,
        'mTR': _f32(mTR), 'mOB': _f32(mOB), 'invA': _f32(invA),
        'endrow': endrow,
    }


# ----------------------------------------------------------------------------
# Device kernel
# ----------------------------------------------------------------------------

PHASE = int(os.environ.get('KPHASE', '9'))


def _build():
    nc = bacc.Bacc("TRN2", target_bir_lowering=False, debug=False,
                   num_devices=NC_)

    d_in = {}
    def din(name, shape, dt):
        d_in[name] = nc.dram_tensor(name, list(shape), dt, kind="ExternalInput").ap()
        return d_in[name]

    emb_d = din('emb', [V, E], BF16)
    xidx_d = din('xidx', [128, NSLOT], I32)
    xridx_d = din('xridx', [128, NSLOT], I32)
    wih_d = din('w_ih', [128, 2 * 2 * 8 * 128], BF16)
    whh_d = din('w_hh', [128, 2 * 2 * 8 * 128], BF16)
    bias_d = din('bias32', [128, 16], F32)
    fcw_d = din('fcw', [128, 4 * K], BF16)
    fcb_d = din('fcbR', [128, K], F32)
    id_d = din('ident', [128, 128], F32)
    idbf_d = din('ident_bf', [128, 128], BF16)
    trans_d = din('transR', [128, K * K], F32)
    iotaD_d = din('iotaD', [128, K], F32)
    iotaK_d = din('iotaK', [128, K], F32)
    validT_d = din('validT', [128, 16], F32)
    ivT_d = din('ivT', [128, 160], F32)
    maskBT_d = din('maskBT', [128, T], F32)
    hboff_d = din('hb_off', [128, NSLOT], I32)
    transNAT_d = din('transNAT', [128, K * K], F32)
    identneg_d = din('identneg', [128, K * K], F32)
    mTR_d = din('mTR', [128, 16], F32)
    mOB_d = din('mOB', [128, 16], F32)
    invA_d = din('invA', [128, 16], F32)
    endrow_d = din('endrow', [BL, 1], I32)

    out_d = nc.dram_tensor('out', [BL, T], F32, kind="ExternalOutput").ap()

    SIG = mybir.ActivationFunctionType.Sigmoid
    TANH = mybir.ActivationFunctionType.Tanh
    AL = mybir.AluOpType
    AX = mybir.AxisListType

    with tile.TileContext(nc) as tc:
        from contextlib import ExitStack
        ctx = ExitStack()
        cpool = ctx.enter_context(tc.tile_pool(name="consts", bufs=1))
        state = ctx.enter_context(tc.tile_pool(name="state", bufs=1))
        gather_p = ctx.enter_context(tc.tile_pool(name="gather", bufs=8))
        scratch = ctx.enter_context(tc.tile_pool(name="scratch", bufs=3))
        vit_p = ctx.enter_context(tc.tile_pool(name="vit", bufs=4))
        vbig = ctx.enter_context(tc.tile_pool(name="vbig", bufs=1))
        ps_tr = ctx.enter_context(tc.tile_pool(name="ps_tr", bufs=2, space="PSUM"))
        ps_mm = ctx.enter_context(tc.tile_pool(name="ps_mm", bufs=2, space="PSUM"))
        ps_g = ctx.enter_context(tc.tile_pool(name="ps_g", bufs=1, space="PSUM"))
        ps_fc = ctx.enter_context(tc.tile_pool(name="ps_fc", bufs=2, space="PSUM"))
        dram_p = ctx.enter_context(tc.tile_pool(name="dram", bufs=1, space="DRAM"))

        hb_dram_t = dram_p.tile([NBT, H], BF16)
        feats_dram_t = dram_p.tile([BL * T * K], F32)
        pre_dram_t = dram_p.tile([BL * T * K], F32)
        bp_dram_t = dram_p.tile([BL, T * K], F32)
        mz_dram_t = dram_p.tile([128 * K * K], F32)
        zb_dram_t = dram_p.tile([BL * 16 * K], F32)
        g_dram_t = dram_p.tile([128 * K], F32)
        r_dram_t = dram_p.tile([128], F32)
        b_dram_t = dram_p.tile([128 * 16], F32)
        hb_dram = hb_dram_t[:]
        feats_dram = feats_dram_t[:]
        pre_dram = pre_dram_t[:]
        bp_dram = bp_dram_t[:]
        mz_dram = mz_dram_t[:]
        zb_dram = zb_dram_t[:]
        g_dram = g_dram_t[:]
        r_dram = r_dram_t[:]
        b_dram = b_dram_t[:]

        def load_const(dram, shape, dt, tag):
            t = cpool.tile(shape, dt, tag=tag)
            nc.sync.dma_start(t[:], dram)
            return t

        # index tensors first: the embedding gathers wait on these
        xidx = load_const(xidx_d[:], [128, NSLOT], I32, tag='xidx')
        xridx = load_const(xridx_d[:], [128, NSLOT], I32, tag='xridx')
        ident_bf = load_const(idbf_d[:], [128, 128], BF16, tag='ident_bf')
        wih = load_const(wih_d[:], [128, 4096], BF16, tag='wih')
        whh = load_const(whh_d[:], [128, 4096], BF16, tag='whh')
        bias = load_const(bias_d[:], [128, 16], F32, tag='bias')
        fcw = load_const(fcw_d[:], [128, 4 * K], BF16, tag='fcw')
        fcbR = load_const(fcb_d[:], [128, K], F32, tag='fcbR')
        ident = load_const(id_d[:], [128, 128], F32, tag='ident')
        transR = load_const(trans_d[:], [128, K * K], F32, tag='transR')
        iotaD = load_const(iotaD_d[:], [128, K], F32, tag='iotaD')
        iotaK = load_const(iotaK_d[:], [128, K], F32, tag='iotaK')
        validT = load_const(validT_d[:], [128, 16], F32, tag='validT')
        ivT = load_const(ivT_d[:], [128, 160], F32, tag='ivT')
        maskBT = load_const(maskBT_d[:], [128, T], F32, tag='maskBT')
        hboff = load_const(hboff_d[:], [128, NSLOT], I32, tag='hboff')
        transNAT = load_const(transNAT_d[:], [128, K * K], F32, tag='transNAT')
        identneg = load_const(identneg_d[:], [128, K * K], F32, tag='identneg')
        mTR = load_const(mTR_d[:], [128, 16], F32, tag='mTR')
        mOB = load_const(mOB_d[:], [128, 16], F32, tag='mOB')
        invA = load_const(invA_d[:], [128, 16], F32, tag='invA')
        endrow = load_const(endrow_d[:], [BL, 1], I32, tag='endrow')

        wih_r = wih[:].rearrange("p (d kc mc m) -> p d kc mc m", d=2, kc=2, mc=8)
        whh_r = whh[:].rearrange("p (d kc mc m) -> p d kc mc m", d=2, kc=2, mc=8)
        fcw_r = fcw[:].rearrange("p (c k) -> p c k", c=4)

        # ---- P1: embedding gather + transpose to x^T (E on partitions) ----
        gx_ctx = ExitStack()
        gxpool = gx_ctx.enter_context(tc.tile_pool(name="gxp", bufs=1))
        x_ctx = ExitStack()
        xpool = x_ctx.enter_context(tc.tile_pool(name="xp", bufs=1))
        x_bf = xpool.tile([128, 2 * 2 * NBT], BF16)   # [p, dir, ec, bt]
        xbf_r = x_bf[:].rearrange("p (d e n) -> p d e n", d=2, e=2)
        for d, idxt in ((0, xidx), (1, xridx)):
            for s_ in range(NSLOT):
                xs = gather_p.tile([128, E], BF16, tag="xslot")
                nc.gpsimd.indirect_dma_start(
                    out=xs[:], out_offset=None, in_=emb_d,
                    in_offset=IndirectOffsetOnAxis(ap=idxt[:, s_:s_ + 1], axis=0),
                )
                for ec in range(2):
                    pt = ps_tr.tile([128, 128], BF16, tag="ptr")
                    nc.tensor.transpose(out=pt[:], in_=xs[:, ec * 128:(ec + 1) * 128],
                                        identity=ident_bf[:])
                    nc.vector.tensor_copy(
                        out=xbf_r[:, d, ec, s_ * 128:(s_ + 1) * 128], in_=pt[:])

        # ---- P2: gx = x @ Wih^T + bias (both dirs), bf16 store ----
        gx = gxpool.tile([128, 2 * 8 * NBT], BF16)     # [p, dir, mc, bt]
        gx_r = gx[:].rearrange("p (d mc n) -> p d mc n", d=2, mc=8)
        gx_rt = gx[:].rearrange("p (d mc b t) -> p d mc b t", d=2, mc=8, b=BL)
        NB = NBT // 512
        for d in range(2):
            for mc in range(8):
                for nb in range(NB):
                    pm = ps_mm.tile([128, 512], F32, tag="pmm")
                    for kc in range(2):
                        nc.tensor.matmul(
                            out=pm[:], lhsT=wih_r[:, d, kc, mc, :],
                            rhs=xbf_r[:, d, kc, nb * 512:(nb + 1) * 512],
                            start=(kc == 0), stop=(kc == 1))
                    nc.vector.tensor_scalar(
                        out=gx_r[:, d, mc, nb * 512:(nb + 1) * 512], in0=pm[:],
                        scalar1=bias[:, d * 8 + mc:d * 8 + mc + 1], scalar2=None,
                        op0=AL.add)

        x_ctx.close()

        # ---- P3: LSTM scan, two independent direction chains interleaved ----
        # Per-dir tiles so the chains share no state: PE does chain d's
        # matmuls while DVE/ACT run the other chain's nonlinearity.
        halls = []
        for d in range(2):
            h_d = state.tile([128, 2 * (T + 1) * BL], BF16,   # [p, kc, t, b]
                             tag=f"hall{d}", name=f"hall{d}")
            halls.append(h_d[:].rearrange("p (kc t b) -> p kc t b", kc=2, t=T + 1))
            nc.vector.memset(halls[d][:, :, 0, :], 0.0)
        cprev = [None, None]
        for d in range(2):
            c0 = scratch.tile([128, 2 * BL], F32, tag=f"c{d}")
            nc.vector.memset(c0[:], 0.0)
            cprev[d] = c0

        # per chain-step (state c2 = 2c, h stored as h/2):
        #   pg   = gx (identity matmul) + sum_kc Whh.h          [PSUM]
        #   sg   = sigmoid(pg)         (g-rows prescaled: sg_g = sig(2g))
        #   t1h  = (sg_g - 0.5) * sg_i                          [stt]
        #   cf2  = sg_f * c2_prev                               [tt]
        #   c2   = 4*t1h + cf2                                  [stt]
        #   sigc = sigmoid(c2)
        #   h/2  = (sigc - 0.5) * sg_o -> halls bf16            [stt]
        def emit_hb_slot(s_):
            # bwd-dir h slots [1+16s, 16+16s] -> transpose -> DRAM rows
            hbs = gather_p.tile([128, H], BF16, tag="hbs")
            for ec in range(2):
                pt = ps_tr.tile([128, 128], BF16, tag="ptr")
                nc.tensor.transpose(
                    out=pt[:],
                    in_=halls[1][:, ec, 1 + s_ * 16:1 + (s_ + 1) * 16, :],
                    identity=ident_bf[:])
                nc.vector.tensor_copy(out=hbs[:, ec * 128:(ec + 1) * 128],
                                      in_=pt[:])
            nc.sync.dma_start(out=hb_dram[s_ * 128:(s_ + 1) * 128, :], in_=hbs[:])

        pgs = {}

        def emit_idmm(t):
            # gx injection into PSUM; independent of h so it runs on the PE
            # while the previous step's nonlinearity is in flight
            for d in range(2):
                pg = ps_g.tile([128, 8 * BL], F32, tag=f"pg{d}")   # [p, mc, b]
                nc.tensor.matmul(out=pg[:], lhsT=ident_bf[:],
                                 rhs=gx_rt[:, d, :, :, t].rearrange(
                                     "p mc b -> p (mc b)"),
                                 start=True, stop=False)
                pgs[d] = pg

        emit_idmm(0)
        for t in range(T):
            sigs, t1s, cfs, sigcs = ({} for _ in range(4))
            curpg = dict(pgs)
            for d in range(2):
                pg_r = curpg[d][:].rearrange("p (mc b) -> p mc b", mc=8)
                for mc in range(8):
                    for kc in range(2):
                        nc.tensor.matmul(
                            out=pg_r[:, mc, :], lhsT=whh_r[:, d, kc, mc, :],
                            rhs=halls[d][:, kc, t, :],
                            start=False, stop=(mc == 7 and kc == 1))
            for d in range(2):
                sig = scratch.tile([128, 8 * BL], F32, tag=f"sig{d}")
                nc.scalar.activation(out=sig[:], in_=curpg[d][:], func=SIG)
                sigs[d] = sig[:].rearrange("p (c b) -> p c b", c=8)
            for d in range(2):
                t1 = scratch.tile([128, 2 * BL], F32, tag=f"t1{d}")
                nc.vector.scalar_tensor_tensor(
                    out=t1[:].rearrange("p (c b) -> p c b", c=2),
                    in0=sigs[d][:, 6:8, :], scalar=0.5, in1=sigs[d][:, 0:2, :],
                    op0=AL.subtract, op1=AL.mult)
                t1s[d] = t1
            for d in range(2):
                cf = scratch.tile([128, 2 * BL], F32, tag=f"cf{d}")
                nc.vector.tensor_mul(
                    out=cf[:].rearrange("p (c b) -> p c b", c=2),
                    in0=sigs[d][:, 2:4, :],
                    in1=cprev[d][:].rearrange("p (c b) -> p c b", c=2))
                cfs[d] = cf
            for d in range(2):
                cn = scratch.tile([128, 2 * BL], F32, tag=f"c{d}")
                nc.vector.scalar_tensor_tensor(
                    out=cn[:], in0=t1s[d][:], scalar=4.0, in1=cfs[d][:],
                    op0=AL.mult, op1=AL.add)
                cprev[d] = cn
            for d in range(2):
                sigc = scratch.tile([128, 2 * BL], F32, tag=f"sigc{d}")
                nc.scalar.activation(out=sigc[:], in_=cprev[d][:], func=SIG)
                sigcs[d] = sigc
            for d in range(2):
                nc.vector.scalar_tensor_tensor(
                    out=halls[d][:, :, t + 1, :],
                    in0=sigcs[d][:].rearrange("p (c b) -> p c b", c=2),
                    scalar=0.5, in1=sigs[d][:, 4:6, :],
                    op0=AL.subtract, op1=AL.mult)
            if t + 1 < T:
                emit_idmm(t + 1)

        gx_ctx.close()

        # ---- P4: hb re-reversal (DRAM bounce + indirect gather + transpose),
        #          then fc emissions ----
        for s_ in range(NSLOT):
            emit_hb_slot(s_)
        hbT = state.tile([128, 2 * NBT], BF16)   # [p(hid), kc, bt]
        hbT_r = hbT[:].rearrange("p (kc n) -> p kc n", kc=2)
        for s_ in range(NSLOT):
            hs = gather_p.tile([128, H], BF16, tag="hslot")
            nc.gpsimd.indirect_dma_start(
                out=hs[:], out_offset=None, in_=hb_dram,
                in_offset=IndirectOffsetOnAxis(ap=hboff[:, s_:s_ + 1], axis=0))
            for ec in range(2):
                pt = ps_tr.tile([128, 128], BF16, tag="ptr")
                nc.tensor.transpose(out=pt[:], in_=hs[:, ec * 128:(ec + 1) * 128],
                                    identity=ident_bf[:])
                nc.vector.tensor_copy(out=hbT_r[:, ec, s_ * 128:(s_ + 1) * 128], in_=pt[:])

        feats_sb = state.tile([128, 16 * K], F32)   # [p, mt, k], bt = mt*128+p
        feats_r = feats_sb[:].rearrange("p (m k) -> p m k", m=16)
        for mt in range(16):
            b_, th = mt // 2, mt % 2
            pf = ps_fc.tile([128, K], F32, tag="pfc")
            for c4 in range(4):
                if c4 < 2:
                    lhs = halls[0][:, c4, 1 + th * 128:1 + (th + 1) * 128, b_]
                else:
                    lhs = hbT_r[:, c4 - 2, mt * 128:(mt + 1) * 128]
                nc.tensor.matmul(out=pf[:], lhsT=lhs, rhs=fcw_r[:, c4, :],
                                 start=(c4 == 0), stop=(c4 == 3))
            nc.vector.tensor_tensor(out=feats_r[:, mt, :], in0=pf[:],
                                    in1=fcbR[:, :], op=AL.add)

        # relayout feats -> [(b,seg) partitions, (tl, k)] for segmented viterbi
        nc.sync.dma_start(
            out=feats_dram.rearrange("(b th p k) -> p b th k", b=BL, th=2, p=128),
            in_=feats_r[:, :, :].rearrange("p (b th) k -> p b th k", b=BL))
        # partition p = b*16 + seg (seg = th*8 + s2); t = seg*16 + tl
        f8seg = state.tile([128, 16 * K], F32)
        nc.sync.dma_start(
            out=f8seg[:],
            in_=feats_dram.rearrange("(b th s2 tl k) -> (b th s2) (tl k)",
                                     b=BL, th=2, s2=8, tl=16))
        f8_r = f8seg[:].rearrange("p (tl k) -> p tl k", tl=16)

        # ---- P5a: build A_t tiles, natural layout A[p, tl, k(prev), j(cur)] ----
        # A = mTR*trans + mOB*ob(bcast k) + invA*identneg
        NSEG = 16
        apool = ctx.enter_context(tc.tile_pool(name="apool", bufs=1))
        vs_p = ctx.enter_context(tc.tile_pool(name="vs", bufs=2))
        At = apool.tile([128, NSEG * K * K], F32)
        At_r = At[:].rearrange("p (tl k j) -> p tl k j", tl=NSEG, k=K)
        tmpA = vbig.tile([128, NSEG * K * K], F32, tag="tmpA")
        tmpA_r = tmpA[:].rearrange("p (tl k j) -> p tl k j", tl=NSEG, k=K)
        tmpO = vbig.tile([128, NSEG * K], F32, tag="tmpO")
        tmpO_r = tmpO[:].rearrange("p (tl j) -> p tl j", tl=NSEG)
        nc.vector.tensor_tensor(
            out=At_r[:, :, :, :],
            in0=transNAT[:, :].rearrange("p (k j) -> p k j", k=K).unsqueeze(1)
                .broadcast_to((128, NSEG, K, K)),
            in1=mTR[:, :].unsqueeze(2).unsqueeze(3).broadcast_to((128, NSEG, K, K)),
            op=AL.mult)
        nc.vector.tensor_tensor(
            out=tmpA_r[:, :, :, :],
            in0=identneg[:, :].rearrange("p (k j) -> p k j", k=K).unsqueeze(1)
                .broadcast_to((128, NSEG, K, K)),
            in1=invA[:, :].unsqueeze(2).unsqueeze(3).broadcast_to((128, NSEG, K, K)),
            op=AL.mult)
        nc.vector.tensor_add(out=At[:], in0=At[:], in1=tmpA[:])
        nc.vector.tensor_tensor(
            out=tmpO_r[:, :, :], in0=f8_r[:, :, :],
            in1=mOB[:, :].unsqueeze(2).broadcast_to((128, NSEG, K)), op=AL.mult)
        nc.vector.tensor_tensor(
            out=At_r[:, :, :, :], in0=At_r[:, :, :, :],
            in1=tmpO_r[:, :, :].unsqueeze(2).broadcast_to((128, NSEG, K, K)),
            op=AL.add)

        # ---- P5b: phase 1 — per-segment max-plus matrix composition ----
        # M[p, i, k] ; step: M'[i, j] = max_k(M[i, k] + A[tl][k, j])
        Mt = vs_p.tile([128, K * K], F32, tag="Mt")
        nc.vector.tensor_copy(out=Mt[:], in_=At_r[:, 0, :, :])
        for tl in range(1, NSEG):
            sb = vs_p.tile([128, K * K * K], F32, tag="sb")
            sb_r = sb[:].rearrange("p (i j k) -> p i j k", i=K, j=K)
            nc.vector.tensor_tensor(
                out=sb_r[:, :, :, :],
                in0=Mt[:].rearrange("p (i k) -> p i k", i=K).unsqueeze(2)
                    .broadcast_to((128, K, K, K)),
                in1=At_r[:, tl, :, :].rearrange("p k j -> p j k").unsqueeze(1)
                    .broadcast_to((128, K, K, K)),
                op=AL.add)
            Mt = vs_p.tile([128, K * K], F32, tag="Mt")
            nc.vector.tensor_reduce(
                out=Mt[:].rearrange("p (i j) -> p i j", i=K),
                in_=sb_r[:, :, :, :], axis=AX.X, op=AL.max)

        # ---- P5c: boundary pass on 8 partitions: z_s = M_s (x) z_{s-1} ----
        nc.sync.dma_start(
            out=mz_dram.rearrange("(b s ij) -> (b s) ij", b=BL, s=NSEG),
            in_=Mt[:])
        M8 = vbig.tile([128, NSEG * K * K], F32, tag="M8")
        nc.sync.dma_start(out=M8[0:BL, :],
                          in_=mz_dram.rearrange("(b sij) -> b sij", b=BL))
        M8_r = M8[:].rearrange("p (s k j) -> p s k j", s=NSEG, k=K)
        zbuf = vbig.tile([128, NSEG * K], F32, tag="zbuf")
        zbuf_r = zbuf[:].rearrange("p (s k) -> p s k", s=NSEG)
        nc.vector.memset(zbuf[0:BL, :], 0.0)
        for s in range(NSEG - 1):
            s3 = vit_p.tile([128, K * K], F32, tag="s3")
            s3_r = s3[:].rearrange("p (j i) -> p j i", j=K)
            nc.vector.tensor_tensor(
                out=s3_r[0:BL, :, :],
                in0=zbuf_r[0:BL, s, :].unsqueeze(1).broadcast_to((BL, K, K)),
                in1=M8_r[0:BL, s, :, :].rearrange("p k j -> p j k"),
                op=AL.add)
            nc.vector.tensor_reduce(
                out=zbuf_r[0:BL, s + 1, :], in_=s3_r[0:BL, :, :],
                axis=AX.X, op=AL.max)

        # relayout z starts -> [(b,seg) partitions, k]
        nc.sync.dma_start(
            out=zb_dram.rearrange("(b sk) -> b sk", b=BL),
            in_=zbuf[0:BL, :])
        zstart = vs_p.tile([128, K], F32, tag="zstart")
        nc.sync.dma_start(out=zstart[:],
                          in_=zb_dram.rearrange("(p k) -> p k", p=128))

        # ---- P5d: phase 2 — within-segment forward scan, all segs parallel ----
        pre_seg = state.tile([128, NSEG * K], F32)
        pre_r = pre_seg[:].rearrange("p (tl k) -> p tl k", tl=NSEG)
        prev_ap = zstart[:, :]
        for tl in range(NSEG):
            s4 = vit_p.tile([128, K * K], F32, tag="s4")
            s4_r = s4[:].rearrange("p (j k) -> p j k", j=K)
            nc.vector.tensor_tensor(
                out=s4_r[:, :, :],
                in0=prev_ap.unsqueeze(1).broadcast_to((128, K, K)),
                in1=At_r[:, tl, :, :].rearrange("p k j -> p j k"),
                op=AL.add)
            nc.vector.tensor_reduce(
                out=pre_r[:, tl, :], in_=s4_r[:, :, :], axis=AX.X, op=AL.max)
            prev_ap = pre_r[:, tl, :]

        # dump pre -> pre_dram in (b t k) order
        nc.sync.dma_start(
            out=pre_dram.rearrange("(b th s2 tl k) -> (b th s2) (tl k)",
                                   b=BL, th=2, s2=8, tl=16),
            in_=pre_seg[:])

        # ---- P5e: end tag via indirect gather of pre[b, n_b-1, :] ----
        peG = vit_p.tile([128, K], F32, tag="peG")
        nc.gpsimd.indirect_dma_start(
            out=peG[0:BL, :], out_offset=None,
            in_=pre_dram.rearrange("(r k) -> r k", k=K),
            in_offset=IndirectOffsetOnAxis(ap=endrow[:, 0:1], axis=0))
        mvE = vit_p.tile([128, 1], F32, tag="mvE")
        nc.vector.tensor_reduce(out=mvE[0:BL, :], in_=peG[0:BL, :], axis=AX.X, op=AL.max)
        eqE = vit_p.tile([128, K], F32, tag="eqE")
        nc.vector.tensor_tensor(out=eqE[0:BL, :], in0=peG[0:BL, :],
                                in1=mvE[0:BL, :].broadcast_to((BL, K)), op=AL.is_equal)
        nc.vector.tensor_mul(out=eqE[0:BL, :], in0=eqE[0:BL, :], in1=iotaD[0:BL, :])
        endt8 = vit_p.tile([128, 1], F32, tag="endt8")
        nc.vector.tensor_reduce(out=endt8[0:BL, :], in_=eqE[0:BL, :], axis=AX.X, op=AL.max)
        nc.vector.tensor_scalar(out=endt8[0:BL, :], in0=endt8[0:BL, :],
                                scalar1=-1.0, scalar2=9.0, op0=AL.mult, op1=AL.add)

        # ---- P6: batched backpointer extraction (from pre_dram, (b t k)) ----
        preT = vbig.tile([128, 2 * BL * K], F32, tag="preT")
        nc.sync.dma_start(
            out=preT[:].rearrange("p (bh k) -> p bh k", bh=2 * BL),
            in_=pre_dram.rearrange("(b h p k) -> p (b h) k", b=BL, h=2, p=128))

        HB = 2 * BL
        preT_hb = preT[:].rearrange("p (hb k) -> p hb k", k=K)
        sX = vbig.tile([128, 2 * BL * K * K], F32, tag="sX")
        sX_r = sX[:].rearrange("p (hb c q) -> p hb c q", hb=HB, c=K)
        nc.vector.tensor_tensor(
            out=sX_r[:, :, :, :],
            in0=preT_hb.unsqueeze(2).broadcast_to((128, HB, K, K)),
            in1=transR[:, :].rearrange("p (c q) -> p c q", c=K).unsqueeze(1)
                .broadcast_to((128, HB, K, K)),
            op=AL.add)
        mX = vbig.tile([128, 2 * BL * K], F32, tag="mX")
        mX_r = mX[:].rearrange("p (hb c) -> p hb c", hb=HB)
        nc.vector.tensor_reduce(out=mX_r[:, :, :], in_=sX_r[:, :, :, :],
                                axis=AX.X, op=AL.max)
        eq = vbig.tile([128, 2 * BL * K * K], F32, tag="eq")
        eq_r = eq[:].rearrange("p (hb c q) -> p hb c q", hb=HB, c=K)
        nc.vector.tensor_tensor(
            out=eq_r[:, :, :, :], in0=sX_r[:, :, :, :],
            in1=mX_r[:, :, :].unsqueeze(3).broadcast_to((128, HB, K, K)),
            op=AL.is_equal)
        nc.vector.tensor_tensor(
            out=eq_r[:, :, :, :], in0=eq_r[:, :, :, :],
            in1=iotaD[:, :].unsqueeze(1).unsqueeze(1).broadcast_to((128, HB, K, K)),
            op=AL.mult)
        bq = vbig.tile([128, 2 * BL * K], F32, tag="bq")
        bq_r = bq[:].rearrange("p (hb c) -> p hb c", hb=HB)
        nc.vector.tensor_reduce(out=bq_r[:, :, :], in_=eq_r[:, :, :, :],
                                axis=AX.X, op=AL.max)
        # bp = 9 - bq ; then pad override: bp*valid + iota_cur*(1-valid)
        nc.vector.tensor_scalar(out=bq[:], in0=bq[:], scalar1=-1.0, scalar2=9.0,
                                op0=AL.mult, op1=AL.add)
        nc.vector.tensor_tensor(
            out=bq_r[:, :, :], in0=bq_r[:, :, :],
            in1=validT[:, :].unsqueeze(2).broadcast_to((128, HB, K)),
            op=AL.mult)
        nc.vector.tensor_tensor(
            out=bq_r[:, :, :], in0=bq_r[:, :, :],
            in1=ivT[:, :].rearrange("p (hb k) -> p hb k", k=K),
            op=AL.add)
        # bp_dram slot t holds the map f_{t+1} (transition into t+1)
        nc.sync.dma_start(
            out=bp_dram[:, :].rearrange("b (h p k) -> p (b h) k", h=2, p=128),
            in_=bq[:].rearrange("p (bh k) -> p bh k", bh=2 * BL))

        # ---- P7: segmented backtrace ----
        # ftile[p=(b,seg), tl, j] = f at u = seg*16+tl, for tl = 1..15
        # (slot u lives at bp_dram position u-1 = seg*16 + (tl-1))
        ftile = state.tile([128, NSEG * K], F32)
        nc.sync.dma_start(
            out=ftile[:, K:],
            in_=bp_dram[:, :].rearrange(
                "b (th s2 tl k) -> (b th s2) (tl k)", th=2, s2=8, tl=16)[:, 0:150])
        ft_r = ftile[:].rearrange("p (tl j) -> p tl j", tl=NSEG)
        # fend8[b, s-1, j] = f_{16s} (= bp_dram position 16s-1), s = 1..15
        fend8 = vbig.tile([128, 15 * K], F32, tag="fend8")
        nc.sync.dma_start(
            out=fend8[0:BL, :].rearrange("p (s k) -> p s k", s=15),
            in_=bp_dram[:, 150:2550].rearrange("b (s gk) -> b s gk",
                                               s=15)[:, :, 0:K])
        fend8_r = fend8[:].rearrange("p (s k) -> p s k", s=15)

        # phase 1: compose G'_s = f_{16s+1} o ... o f_{16s+15}
        Ct = vs_p.tile([128, K], F32, tag="Ct")
        nc.vector.tensor_copy(out=Ct[:], in_=ft_r[:, NSEG - 1, :])
        for tl in range(NSEG - 2, 0, -1):
            ohB = vit_p.tile([128, K * K], F32, tag="ohB")
            ohB_r = ohB[:].rearrange("p (i j) -> p i j", i=K)
            nc.vector.tensor_tensor(
                out=ohB_r[:, :, :],
                in0=Ct[:].unsqueeze(2).broadcast_to((128, K, K)),
                in1=iotaK[:, :].unsqueeze(1).broadcast_to((128, K, K)),
                op=AL.is_equal)
            nc.vector.tensor_tensor(
                out=ohB_r[:, :, :], in0=ohB_r[:, :, :],
                in1=ft_r[:, tl, :].unsqueeze(1).broadcast_to((128, K, K)),
                op=AL.mult)
            Ct = vs_p.tile([128, K], F32, tag="Ct")
            nc.vector.tensor_reduce(out=Ct[:], in_=ohB_r[:, :, :],
                                    axis=AX.X, op=AL.max)

        # relayout G -> [8, (s, i)]
        nc.sync.dma_start(
            out=g_dram.rearrange("(b s i) -> (b s) i", b=BL, s=NSEG), in_=Ct[:])
        G8 = vbig.tile([128, NSEG * K], F32, tag="G8")
        nc.sync.dma_start(out=G8[0:BL, :],
                          in_=g_dram.rearrange("(b si) -> b si", b=BL))
        G8_r = G8[:].rearrange("p (s i) -> p s i", s=NSEG)

        # boundary pass: r_{s-1} = f_{16s}(G'_s(r_s)), r_15 = end tag
        rbuf = vbig.tile([128, NSEG], F32, tag="rbuf")
        nc.vector.tensor_copy(out=rbuf[0:BL, NSEG - 1:NSEG], in_=endt8[0:BL, :])
        for s in range(NSEG - 1, 0, -1):
            oh8 = vit_p.tile([128, K], F32, tag="oh8")
            nc.vector.tensor_tensor(
                out=oh8[0:BL, :], in0=iotaK[0:BL, :],
                in1=rbuf[0:BL, s:s + 1].broadcast_to((BL, K)), op=AL.is_equal)
            nc.vector.tensor_mul(out=oh8[0:BL, :], in0=oh8[0:BL, :],
                                 in1=G8_r[0:BL, s, :])
            tG = vit_p.tile([128, 1], F32, tag="tG")
            nc.vector.tensor_reduce(out=tG[0:BL, :], in_=oh8[0:BL, :],
                                    axis=AX.X, op=AL.max)
            oh9 = vit_p.tile([128, K], F32, tag="oh9")
            nc.vector.tensor_tensor(
                out=oh9[0:BL, :], in0=iotaK[0:BL, :],
                in1=tG[0:BL, :].broadcast_to((BL, K)), op=AL.is_equal)
            nc.vector.tensor_mul(out=oh9[0:BL, :], in0=oh9[0:BL, :],
                                 in1=fend8_r[0:BL, s - 1, :])
            nc.vector.tensor_reduce(out=rbuf[0:BL, s - 1:s], in_=oh9[0:BL, :],
                                    axis=AX.X, op=AL.max)

        # relayout r -> [(b,seg) partitions, 1]
        nc.sync.dma_start(out=r_dram.rearrange("(b s) -> b s", b=BL),
                          in_=rbuf[0:BL, :])
        rstart = vs_p.tile([128, 1], F32, tag="rstart")
        nc.sync.dma_start(out=rstart[:],
                          in_=r_dram.rearrange("(p one) -> p one", one=1))

        # phase 2: walk back within each segment, all segs parallel
        bestseg = state.tile([128, NSEG], F32)
        nc.vector.tensor_copy(out=bestseg[:, NSEG - 1:NSEG], in_=rstart[:])
        for tl in range(NSEG - 1, 0, -1):
            oh2 = vit_p.tile([128, K], F32, tag="oh2")
            nc.vector.tensor_tensor(
                out=oh2[:, :], in0=iotaK[:, :],
                in1=bestseg[:, tl:tl + 1].broadcast_to((128, K)), op=AL.is_equal)
            nc.vector.tensor_mul(out=oh2[:, :], in0=oh2[:, :], in1=ft_r[:, tl, :])
            nc.vector.tensor_reduce(out=bestseg[:, tl - 1:tl], in_=oh2[:, :],
                                    axis=AX.X, op=AL.max)

        # bestseg[p=(b,seg), tl] -> [8, 256], mask, out
        nc.sync.dma_start(
            out=b_dram.rearrange("(b th s2 tl) -> (b th s2) tl",
                                 b=BL, th=2, s2=8),
            in_=bestseg[:])
        best8 = state.tile([128, T], F32)
        nc.sync.dma_start(out=best8[0:BL, :],
                          in_=b_dram.rearrange("(b t) -> b t", b=BL))
        bestM = state.tile([128, T], F32)
        nc.vector.tensor_mul(out=bestM[0:BL, :], in0=best8[0:BL, :],
                             in1=maskBT[0:BL, :])
        nc.sync.dma_start(out=out_d, in_=bestM[0:BL, :])
        ctx.close()

    nc.compile()
    return nc


_NC_CACHE = None


def _get_nc():
    global _NC_CACHE
    if _NC_CACHE is None:
        _NC_CACHE = _build()
    return _NC_CACHE


TRACE = False
LAST_EXEC_NS = None


def kernel(**inputs) -> np.ndarray:
    global LAST_EXEC_NS
    nc = _get_nc()
    in_maps = [_prep_core(inputs, c) for c in range(NC_)]
    res = run_bass_kernel_spmd(nc, in_maps, list(range(NC_)), trace=TRACE)
    LAST_EXEC_NS = res.exec_time_ns
    out = np.concatenate([res.results[c]['out'] for c in range(NC_)], axis=0)
    return out.astype(np.float32)


if __name__ == '__main__':
    _build()
    print("build ok")



# revision 61
# speedup vs baseline: 1.0756x; 1.0285x over previous
"""BiLSTM-CRF (Viterbi decode) Trainium2 Bass kernel, 8-core data-parallel.

Full inputs in, full outputs out. Batch (64) is sharded 8 ways; each core runs:
  embedding gather -> input matmuls (gx = x @ Wih^T + b) -> 256-step fused
  fwd+bwd LSTM recurrence -> fc emissions -> Viterbi scan -> batched
  backpointer extraction -> backtrace.

Layout convention on device ("version B"): gate/hidden dims live on SBUF
partitions, batch on the free dim, so ACT/DVE use all 128 lanes.
"""

import os
import sys
import types

for _p in ('/opt/trn_rl_repo', '/root/.axon_site'):
    if _p not in sys.path:
        sys.path.insert(0, _p)

import numpy as np
import ml_dtypes

# ---- NTFF profile hook (lets run_bass_kernel_spmd(trace=True) return timings
# under axon; harmless if already registered or unavailable) ----
def _install_ntff_hook():
    try:
        import antenv
        if 'antenv.axon_hooks' in sys.modules:
            return
        from trn_agent_boot.trn_boot import _ntff_profile_via_ctypes
        m = types.ModuleType('antenv.axon_hooks')
        m._hook = _ntff_profile_via_ctypes('/opt/axon/libaxon_pjrt.so')
        m.get_axon_ntff_profile_hook = lambda: m._hook
        m.set_axon_ntff_profile_hook = lambda h: setattr(m, '_hook', h)
        sys.modules['antenv.axon_hooks'] = m
        antenv.axon_hooks = m
    except Exception:
        pass


_install_ntff_hook()

import concourse.bass as bass
import concourse.tile as tile
from concourse import bacc, mybir
from concourse.bass import IndirectOffsetOnAxis
from concourse.bass_utils import run_bass_kernel_spmd

F32 = mybir.dt.float32
BF16 = mybir.dt.bfloat16
F8 = mybir.dt.float8e4
F16 = mybir.dt.float16
I32 = mybir.dt.int32

# Problem dims (hardcoded per contract)
V, E, HS, T, B = 30000, 256, 512, 256, 64
H = HS // 2          # 256 per-direction hidden
G = 4 * H            # 1024 gate rows per direction
K = 10               # tags
NC_ = 8              # cores
BL = B // NC_        # 8 sequences per core
NBT = BL * T         # 2048 (b,t) columns per core
NSLOT = NBT // 128   # 16 gather slots

# Gate reorder: torch rows [i, f, g, o] -> device order [i, f, o, g]
# (sigmoid block = chunks 0..5, tanh block = chunks 6..7)
_PERM = np.concatenate([
    np.arange(0, 2 * H),          # i, f
    np.arange(3 * H, 4 * H),      # o
    np.arange(2 * H, 3 * H),      # g
])


def _bf(x):
    return np.ascontiguousarray(np.asarray(x, np.float32).astype(ml_dtypes.bfloat16))


def _f32(x):
    return np.ascontiguousarray(np.asarray(x, np.float32))


def _pack_w(wih, whh, bih, bhh):
    """Per direction: returns (w_ih[128, 2*8*128], w_hh[...], bias[128, 8]) in
    lhsT tile layout w[p, kc, mc, m] = W[perm[mc*128+m], kc*128+p].
    Scalings: h is stored as h/2 on device (so Whh gets *2), and g-gate
    rows (last H after perm) are pre-scaled by 2 so sigmoid gives
    tanh(g) = 2*sig(2g) - 1."""
    out = []
    for W, hscale in ((wih, 1.0), (whh, 2.0)):
        Wp = np.asarray(W, np.float32)[_PERM] * hscale  # [G, Kdim]
        Wp[3 * H:] *= 2.0
        Kd = Wp.shape[1]
        t = Wp.reshape(8, 128, Kd // 128, 128)          # [mc, m, kc, p]
        t = np.transpose(t, (3, 2, 0, 1))               # [p, kc, mc, m]
        out.append(t.reshape(128, -1))
    b = (np.asarray(bih, np.float32) + np.asarray(bhh, np.float32))[_PERM].copy()
    b[3 * H:] *= 2.0
    b = b.reshape(8, 128).T                             # [p, mc]
    return out[0], out[1], b


def _prep_core(inputs, core):
    """Host-side prep of all per-core device inputs."""
    s = slice(core * BL, (core + 1) * BL)
    inp = np.asarray(inputs['inp'])[s]        # [8, 256] int
    n = np.asarray(inputs['n'])[s].astype(np.int64)

    t_idx = np.arange(T)
    mask = t_idx[None, :] < n[:, None]
    rev = np.where(mask, n[:, None] - 1 - t_idx[None, :], t_idx[None, :])
    tok_rev = np.take_along_axis(inp, rev, axis=1)

    def idx_pack(tok):  # [8,256] -> [128, 16] slot layout (j = s*128+p, j=b*256+t)
        flat = np.asarray(tok, np.int64).reshape(-1)     # j = b*256+t
        return flat.reshape(NSLOT, 128).T.astype(np.int32).copy()

    wf = _pack_w(inputs['W_ih_f'], inputs['W_hh_f'], inputs['b_ih_f'], inputs['b_hh_f'])
    wb = _pack_w(inputs['W_ih_b'], inputs['W_hh_b'], inputs['b_ih_b'], inputs['b_hh_b'])
    w_ih = _bf(np.concatenate([wf[0], wb[0]], axis=1))   # [128, 2*2048]
    w_hh = _bf(np.concatenate([wf[1], wb[1]], axis=1))
    bias = _f32(np.concatenate([wf[2], wb[2]], axis=1))  # [128, 16] (d, mc)

    fcw = np.asarray(inputs['fc_w'], np.float32) * 2.0   # [10, 512]; h stored as h/2
    fcw_t = fcw.T.reshape(4, 128, K).transpose(1, 0, 2).reshape(128, 4 * K)
    fcbR = np.tile(np.asarray(inputs['fc_b'], np.float32)[None, :], (128, 1))

    trans = np.asarray(inputs['transition'], np.float32)[:K, :K]  # [prev, cur]
    transR = np.tile(trans.T.reshape(1, K * K), (128, 1)).copy()  # [p, cur*10+prev]

    iotaD = np.tile((9.0 - np.arange(K, dtype=np.float32))[None, :], (128, 1))
    iotaK = np.tile(np.arange(K, dtype=np.float32)[None, :], (128, 1))

    # validT[p, b*2+h] = ((h*128+p)+1 < n_b); ivT[p, (b,h,k)] = k*(1-valid)
    tt = (np.arange(256).reshape(2, 128).T)[:, None, :]            # [p, 1, h]
    validT = (tt + 1 < n[None, :, None]).astype(np.float32)        # [p, b, h]
    ivT = (1.0 - validT)[:, :, :, None] * np.arange(K, dtype=np.float32)[None, None, None, :]

    maskBT = np.zeros((128, T), np.float32)
    maskBT[:BL] = mask.astype(np.float32)

    # --- segmented viterbi/backtrace constants (partition p = b*16 + seg) ---
    transNAT = np.tile(trans.reshape(1, K * K), (128, 1)).copy()   # [p, k*10+j]
    idn = np.full((K, K), -1e9, np.float32)
    np.fill_diagonal(idn, 0.0)
    identneg = np.tile(idn.reshape(1, K * K), (128, 1)).copy()
    pb = np.arange(128) // 16                                       # seq of partition
    pseg = np.arange(128) % 16
    tgrid = pseg[:, None] * 16 + np.arange(16)[None, :]             # [p, tl] -> t
    mOB = (tgrid < n[pb][:, None]).astype(np.float32)               # emission valid
    mTR = mOB * (tgrid != 0)                                        # transition valid
    invA = 1.0 - mOB
    endrow = (np.arange(BL) * T + (n - 1)).astype(np.int32).reshape(BL, 1)

    # hb re-reversal gather rows: out col j=(b,t) <- hb_dram row b*256 + scan_idx
    scan_idx = np.where(mask, n[:, None] - 1 - t_idx[None, :], t_idx[None, :])
    hb_rows = ((scan_idx // 16) * 128 + (scan_idx % 16) * 8
               + np.arange(BL)[:, None]).reshape(-1)
    hb_off = hb_rows.reshape(NSLOT, 128).T.astype(np.int32).copy()

    return {
        'emb': _bf(inputs['emb']),
        'xidx': idx_pack(inp),
        'xridx': idx_pack(tok_rev),
        'w_ih': w_ih, 'w_hh': w_hh, 'bias32': bias,
        'fcw': _bf(fcw_t), 'fcbR': fcbR,
        'ident': np.eye(128, dtype=np.float32),
        'ident_bf': _bf(np.eye(128, dtype=np.float32)),
        'transR': transR, 'iotaD': iotaD, 'iotaK': iotaK,
        'validT': _f32(validT.reshape(128, 16)),
        'ivT': _f32(ivT.reshape(128, 160)),
        'maskBT': maskBT,
        'hb_off': hb_off,
        'transNAT': transNAT, 'identneg': identneg,
        'mTR': _f32(mTR), 'mOB': _f32(mOB), 'invA': _f32(invA),
        'endrow': endrow,
    }


# ----------------------------------------------------------------------------
# Device kernel
# ----------------------------------------------------------------------------

PHASE = int(os.environ.get('KPHASE', '9'))


def _build():
    nc = bacc.Bacc("TRN2", target_bir_lowering=False, debug=False,
                   num_devices=NC_)

    d_in = {}
    def din(name, shape, dt):
        d_in[name] = nc.dram_tensor(name, list(shape), dt, kind="ExternalInput").ap()
        return d_in[name]

    emb_d = din('emb', [V, E], BF16)
    xidx_d = din('xidx', [128, NSLOT], I32)
    xridx_d = din('xridx', [128, NSLOT], I32)
    wih_d = din('w_ih', [128, 2 * 2 * 8 * 128], BF16)
    whh_d = din('w_hh', [128, 2 * 2 * 8 * 128], BF16)
    bias_d = din('bias32', [128, 16], F32)
    fcw_d = din('fcw', [128, 4 * K], BF16)
    fcb_d = din('fcbR', [128, K], F32)
    id_d = din('ident', [128, 128], F32)
    idbf_d = din('ident_bf', [128, 128], BF16)
    trans_d = din('transR', [128, K * K], F32)
    iotaD_d = din('iotaD', [128, K], F32)
    iotaK_d = din('iotaK', [128, K], F32)
    validT_d = din('validT', [128, 16], F32)
    ivT_d = din('ivT', [128, 160], F32)
    maskBT_d = din('maskBT', [128, T], F32)
    hboff_d = din('hb_off', [128, NSLOT], I32)
    transNAT_d = din('transNAT', [128, K * K], F32)
    identneg_d = din('identneg', [128, K * K], F32)
    mTR_d = din('mTR', [128, 16], F32)
    mOB_d = din('mOB', [128, 16], F32)
    invA_d = din('invA', [128, 16], F32)
    endrow_d = din('endrow', [BL, 1], I32)

    out_d = nc.dram_tensor('out', [BL, T], F32, kind="ExternalOutput").ap()

    SIG = mybir.ActivationFunctionType.Sigmoid
    TANH = mybir.ActivationFunctionType.Tanh
    AL = mybir.AluOpType
    AX = mybir.AxisListType

    with tile.TileContext(nc) as tc:
        from contextlib import ExitStack
        ctx = ExitStack()
        cpool = ctx.enter_context(tc.tile_pool(name="consts", bufs=1))
        state = ctx.enter_context(tc.tile_pool(name="state", bufs=1))
        gather_p = ctx.enter_context(tc.tile_pool(name="gather", bufs=8))
        scratch = ctx.enter_context(tc.tile_pool(name="scratch", bufs=3))
        vit_p = ctx.enter_context(tc.tile_pool(name="vit", bufs=4))
        vbig = ctx.enter_context(tc.tile_pool(name="vbig", bufs=1))
        ps_tr = ctx.enter_context(tc.tile_pool(name="ps_tr", bufs=2, space="PSUM"))
        ps_mm = ctx.enter_context(tc.tile_pool(name="ps_mm", bufs=2, space="PSUM"))
        ps_g = ctx.enter_context(tc.tile_pool(name="ps_g", bufs=1, space="PSUM"))
        ps_fc = ctx.enter_context(tc.tile_pool(name="ps_fc", bufs=2, space="PSUM"))
        dram_p = ctx.enter_context(tc.tile_pool(name="dram", bufs=1, space="DRAM"))

        hb_dram_t = dram_p.tile([NBT, H], BF16)
        feats_dram_t = dram_p.tile([BL * T * K], F32)
        pre_dram_t = dram_p.tile([BL * T * K], F32)
        bp_dram_t = dram_p.tile([BL, T * K], F32)
        mz_dram_t = dram_p.tile([128 * K * K], F32)
        zb_dram_t = dram_p.tile([BL * 16 * K], F32)
        g_dram_t = dram_p.tile([128 * K], F32)
        r_dram_t = dram_p.tile([128], F32)
        b_dram_t = dram_p.tile([128 * 16], F32)
        hb_dram = hb_dram_t[:]
        feats_dram = feats_dram_t[:]
        pre_dram = pre_dram_t[:]
        bp_dram = bp_dram_t[:]
        mz_dram = mz_dram_t[:]
        zb_dram = zb_dram_t[:]
        g_dram = g_dram_t[:]
        r_dram = r_dram_t[:]
        b_dram = b_dram_t[:]

        def load_const(dram, shape, dt, tag):
            t = cpool.tile(shape, dt, tag=tag)
            nc.sync.dma_start(t[:], dram)
            return t

        # index tensors first: the embedding gathers wait on these
        xidx = load_const(xidx_d[:], [128, NSLOT], I32, tag='xidx')
        xridx = load_const(xridx_d[:], [128, NSLOT], I32, tag='xridx')
        ident_bf = load_const(idbf_d[:], [128, 128], BF16, tag='ident_bf')
        wih = load_const(wih_d[:], [128, 4096], BF16, tag='wih')
        whh = load_const(whh_d[:], [128, 4096], BF16, tag='whh')
        bias = load_const(bias_d[:], [128, 16], F32, tag='bias')
        fcw = load_const(fcw_d[:], [128, 4 * K], BF16, tag='fcw')
        fcbR = load_const(fcb_d[:], [128, K], F32, tag='fcbR')
        ident = load_const(id_d[:], [128, 128], F32, tag='ident')
        transR = load_const(trans_d[:], [128, K * K], F32, tag='transR')
        iotaD = load_const(iotaD_d[:], [128, K], F32, tag='iotaD')
        iotaK = load_const(iotaK_d[:], [128, K], F32, tag='iotaK')
        validT = load_const(validT_d[:], [128, 16], F32, tag='validT')
        ivT = load_const(ivT_d[:], [128, 160], F32, tag='ivT')
        maskBT = load_const(maskBT_d[:], [128, T], F32, tag='maskBT')
        hboff = load_const(hboff_d[:], [128, NSLOT], I32, tag='hboff')
        transNAT = load_const(transNAT_d[:], [128, K * K], F32, tag='transNAT')
        identneg = load_const(identneg_d[:], [128, K * K], F32, tag='identneg')
        mTR = load_const(mTR_d[:], [128, 16], F32, tag='mTR')
        mOB = load_const(mOB_d[:], [128, 16], F32, tag='mOB')
        invA = load_const(invA_d[:], [128, 16], F32, tag='invA')
        endrow = load_const(endrow_d[:], [BL, 1], I32, tag='endrow')

        wih_r = wih[:].rearrange("p (d kc mc m) -> p d kc mc m", d=2, kc=2, mc=8)
        whh_r = whh[:].rearrange("p (d kc mc m) -> p d kc mc m", d=2, kc=2, mc=8)
        fcw_r = fcw[:].rearrange("p (c k) -> p c k", c=4)

        # ---- P1: embedding gather + transpose to x^T (E on partitions) ----
        gx_ctx = ExitStack()
        gxpool = gx_ctx.enter_context(tc.tile_pool(name="gxp", bufs=1))
        x_ctx = ExitStack()
        xpool = x_ctx.enter_context(tc.tile_pool(name="xp", bufs=1))
        x_bf = xpool.tile([128, 2 * 2 * NBT], BF16)   # [p, dir, ec, bt]
        xbf_r = x_bf[:].rearrange("p (d e n) -> p d e n", d=2, e=2)
        for d, idxt in ((0, xidx), (1, xridx)):
            for s_ in range(NSLOT):
                xs = gather_p.tile([128, E], BF16, tag="xslot")
                nc.gpsimd.indirect_dma_start(
                    out=xs[:], out_offset=None, in_=emb_d,
                    in_offset=IndirectOffsetOnAxis(ap=idxt[:, s_:s_ + 1], axis=0),
                )
                for ec in range(2):
                    pt = ps_tr.tile([128, 128], BF16, tag="ptr")
                    nc.tensor.transpose(out=pt[:], in_=xs[:, ec * 128:(ec + 1) * 128],
                                        identity=ident_bf[:])
                    nc.vector.tensor_copy(
                        out=xbf_r[:, d, ec, s_ * 128:(s_ + 1) * 128], in_=pt[:])

        # ---- P2: gx = x @ Wih^T + bias (both dirs), bf16 store ----
        gx = gxpool.tile([128, 2 * 8 * NBT], BF16)     # [p, dir, mc, bt]
        gx_r = gx[:].rearrange("p (d mc n) -> p d mc n", d=2, mc=8)
        gx_rt = gx[:].rearrange("p (d mc b t) -> p d mc b t", d=2, mc=8, b=BL)
        NB = NBT // 512
        for d in range(2):
            for mc in range(8):
                for nb in range(NB):
                    pm = ps_mm.tile([128, 512], F32, tag="pmm")
                    for kc in range(2):
                        nc.tensor.matmul(
                            out=pm[:], lhsT=wih_r[:, d, kc, mc, :],
                            rhs=xbf_r[:, d, kc, nb * 512:(nb + 1) * 512],
                            start=(kc == 0), stop=(kc == 1))
                    nc.vector.tensor_scalar(
                        out=gx_r[:, d, mc, nb * 512:(nb + 1) * 512], in0=pm[:],
                        scalar1=bias[:, d * 8 + mc:d * 8 + mc + 1], scalar2=None,
                        op0=AL.add)

        x_ctx.close()

        # ---- P3: LSTM scan, two independent direction chains interleaved ----
        # Per-dir tiles so the chains share no state: PE does chain d's
        # matmuls while DVE/ACT run the other chain's nonlinearity.
        halls = []
        for d in range(2):
            h_d = state.tile([128, 2 * (T + 1) * BL], BF16,   # [p, kc, t, b]
                             tag=f"hall{d}", name=f"hall{d}")
            halls.append(h_d[:].rearrange("p (kc t b) -> p kc t b", kc=2, t=T + 1))
            nc.vector.memset(halls[d][:, :, 0, :], 0.0)
        cprev = [None, None]
        for d in range(2):
            c0 = scratch.tile([128, 2 * BL], F32, tag=f"c{d}")
            nc.vector.memset(c0[:], 0.0)
            cprev[d] = c0

        # per chain-step (state c2 = 2c, h stored as h/2):
        #   pg   = gx (identity matmul) + sum_kc Whh.h          [PSUM]
        #   sg   = sigmoid(pg)         (g-rows prescaled: sg_g = sig(2g))
        #   t1h  = (sg_g - 0.5) * sg_i                          [stt]
        #   cf2  = sg_f * c2_prev                               [tt]
        #   c2   = 4*t1h + cf2                                  [stt]
        #   sigc = sigmoid(c2)
        #   h/2  = (sigc - 0.5) * sg_o -> halls bf16            [stt]
        def emit_hb_slot(s_):
            # bwd-dir h slots [1+16s, 16+16s] -> transpose -> DRAM rows
            hbs = gather_p.tile([128, H], BF16, tag="hbs")
            for ec in range(2):
                pt = ps_tr.tile([128, 128], BF16, tag="ptr")
                nc.tensor.transpose(
                    out=pt[:],
                    in_=halls[1][:, ec, 1 + s_ * 16:1 + (s_ + 1) * 16, :],
                    identity=ident_bf[:])
                nc.vector.tensor_copy(out=hbs[:, ec * 128:(ec + 1) * 128],
                                      in_=pt[:])
            nc.sync.dma_start(out=hb_dram[s_ * 128:(s_ + 1) * 128, :], in_=hbs[:])

        pgs = {}

        def emit_idmm(t):
            # gx injection into PSUM; independent of h so it runs on the PE
            # while the previous step's nonlinearity is in flight
            for d in range(2):
                pg = ps_g.tile([128, 8 * BL], F32, tag=f"pg{d}")   # [p, mc, b]
                nc.tensor.matmul(out=pg[:], lhsT=ident_bf[:],
                                 rhs=gx_rt[:, d, :, :, t].rearrange(
                                     "p mc b -> p (mc b)"),
                                 start=True, stop=False)
                pgs[d] = pg

        emit_idmm(0)
        for t in range(T):
            sigs, t1s, cfs, sigcs = ({} for _ in range(4))
            curpg = dict(pgs)
            for d in range(2):
                pg_r = curpg[d][:].rearrange("p (mc b) -> p mc b", mc=8)
                for mc in range(8):
                    for kc in range(2):
                        nc.tensor.matmul(
                            out=pg_r[:, mc, :], lhsT=whh_r[:, d, kc, mc, :],
                            rhs=halls[d][:, kc, t, :],
                            start=False, stop=(mc == 7 and kc == 1))
            for d in range(2):
                sig = scratch.tile([128, 8 * BL], F32, tag=f"sig{d}")
                nc.scalar.activation(out=sig[:], in_=curpg[d][:], func=SIG)
                sigs[d] = sig[:].rearrange("p (c b) -> p c b", c=8)
            for d in range(2):
                t1 = scratch.tile([128, 2 * BL], F32, tag=f"t1{d}")
                nc.vector.scalar_tensor_tensor(
                    out=t1[:].rearrange("p (c b) -> p c b", c=2),
                    in0=sigs[d][:, 6:8, :], scalar=0.5, in1=sigs[d][:, 0:2, :],
                    op0=AL.subtract, op1=AL.mult)
                t1s[d] = t1
            for d in range(2):
                cf = scratch.tile([128, 2 * BL], F32, tag=f"cf{d}")
                nc.vector.tensor_mul(
                    out=cf[:].rearrange("p (c b) -> p c b", c=2),
                    in0=sigs[d][:, 2:4, :],
                    in1=cprev[d][:].rearrange("p (c b) -> p c b", c=2))
                cfs[d] = cf
            for d in range(2):
                cn = scratch.tile([128, 2 * BL], F32, tag=f"c{d}")
                nc.vector.scalar_tensor_tensor(
                    out=cn[:], in0=t1s[d][:], scalar=4.0, in1=cfs[d][:],
                    op0=AL.mult, op1=AL.add)
                cprev[d] = cn
            for d in range(2):
                sigc = scratch.tile([128, 2 * BL], F32, tag=f"sigc{d}")
                nc.scalar.activation(out=sigc[:], in_=cprev[d][:], func=SIG)
                sigcs[d] = sigc
            for d in range(2):
                nc.vector.scalar_tensor_tensor(
                    out=halls[d][:, :, t + 1, :],
                    in0=sigcs[d][:].rearrange("p (c b) -> p c b", c=2),
                    scalar=0.5, in1=sigs[d][:, 4:6, :],
                    op0=AL.subtract, op1=AL.mult)
            if t + 1 < T:
                emit_idmm(t + 1)

        gx_ctx.close()

        # ---- P4: hb re-reversal (DRAM bounce + indirect gather + transpose),
        #          then fc emissions ----
        for s_ in range(NSLOT):
            emit_hb_slot(s_)
        hbT = state.tile([128, 2 * NBT], BF16)   # [p(hid), kc, bt]
        hbT_r = hbT[:].rearrange("p (kc n) -> p kc n", kc=2)
        for s_ in range(NSLOT):
            hs = gather_p.tile([128, H], BF16, tag="hslot")
            nc.gpsimd.indirect_dma_start(
                out=hs[:], out_offset=None, in_=hb_dram,
                in_offset=IndirectOffsetOnAxis(ap=hboff[:, s_:s_ + 1], axis=0))
            for ec in range(2):
                pt = ps_tr.tile([128, 128], BF16, tag="ptr")
                nc.tensor.transpose(out=pt[:], in_=hs[:, ec * 128:(ec + 1) * 128],
                                    identity=ident_bf[:])
                nc.vector.tensor_copy(out=hbT_r[:, ec, s_ * 128:(s_ + 1) * 128], in_=pt[:])

        feats_sb = state.tile([128, 16 * K], F32)   # [p, mt, k], bt = mt*128+p
        feats_r = feats_sb[:].rearrange("p (m k) -> p m k", m=16)
        for mt in range(16):
            b_, th = mt // 2, mt % 2
            pf = ps_fc.tile([128, K], F32, tag="pfc")
            for c4 in range(4):
                if c4 < 2:
                    lhs = halls[0][:, c4, 1 + th * 128:1 + (th + 1) * 128, b_]
                else:
                    lhs = hbT_r[:, c4 - 2, mt * 128:(mt + 1) * 128]
                nc.tensor.matmul(out=pf[:], lhsT=lhs, rhs=fcw_r[:, c4, :],
                                 start=(c4 == 0), stop=(c4 == 3))
            nc.vector.tensor_tensor(out=feats_r[:, mt, :], in0=pf[:],
                                    in1=fcbR[:, :], op=AL.add)

        # relayout feats -> [(b,seg) partitions, (tl, k)] for segmented viterbi
        nc.sync.dma_start(
            out=feats_dram.rearrange("(b th p k) -> p b th k", b=BL, th=2, p=128),
            in_=feats_r[:, :, :].rearrange("p (b th) k -> p b th k", b=BL))
        # partition p = b*16 + seg (seg = th*8 + s2); t = seg*16 + tl
        f8seg = state.tile([128, 16 * K], F32)
        nc.sync.dma_start(
            out=f8seg[:],
            in_=feats_dram.rearrange("(b th s2 tl k) -> (b th s2) (tl k)",
                                     b=BL, th=2, s2=8, tl=16))
        f8_r = f8seg[:].rearrange("p (tl k) -> p tl k", tl=16)

        # ---- P5a: build A_t tiles in transposed layout A2[p, tl, j(cur),
        #      k(prev)] = A[k, j], bf16, so phase-1 reads are stride-1 (2x) ----
        NSEG = 16
        apool = ctx.enter_context(tc.tile_pool(name="apool", bufs=1))
        vs_p = ctx.enter_context(tc.tile_pool(name="vs", bufs=2))
        At = apool.tile([128, NSEG * K * K], F32)
        At_r = At[:].rearrange("p (tl j k) -> p tl j k", tl=NSEG, j=K)
        tmpA = vbig.tile([128, NSEG * K * K], F32, tag="tmpA")
        tmpA_r = tmpA[:].rearrange("p (tl j k) -> p tl j k", tl=NSEG, j=K)
        tmpO = vbig.tile([128, NSEG * K], F32, tag="tmpO")
        tmpO_r = tmpO[:].rearrange("p (tl j) -> p tl j", tl=NSEG)
        nc.vector.tensor_tensor(
            out=tmpA_r[:, :, :, :],
            in0=transR[:, :].rearrange("p (j k) -> p j k", j=K).unsqueeze(1)
                .broadcast_to((128, NSEG, K, K)),
            in1=mTR[:, :].unsqueeze(2).unsqueeze(3).broadcast_to((128, NSEG, K, K)),
            op=AL.mult)
        nc.vector.tensor_tensor(
            out=tmpO_r[:, :, :], in0=f8_r[:, :, :],
            in1=mOB[:, :].unsqueeze(2).broadcast_to((128, NSEG, K)), op=AL.mult)
        nc.vector.tensor_tensor(
            out=tmpA_r[:, :, :, :], in0=tmpA_r[:, :, :, :],
            in1=tmpO_r[:, :, :].unsqueeze(3).broadcast_to((128, NSEG, K, K)),
            op=AL.add)
        tmpB = vbig.tile([128, NSEG * K * K], F32, tag="tmpB")
        tmpB_r = tmpB[:].rearrange("p (tl j k) -> p tl j k", tl=NSEG, j=K)
        nc.vector.tensor_tensor(
            out=tmpB_r[:, :, :, :],
            in0=identneg[:, :].rearrange("p (j k) -> p j k", j=K).unsqueeze(1)
                .broadcast_to((128, NSEG, K, K)),
            in1=invA[:, :].unsqueeze(2).unsqueeze(3).broadcast_to((128, NSEG, K, K)),
            op=AL.mult)
        nc.vector.tensor_tensor(
            out=At_r[:, :, :, :], in0=tmpA_r[:, :, :, :],
            in1=tmpB_r[:, :, :, :], op=AL.add)

        # ---- P5b: phase 1 — per-segment max-plus matrix composition ----
        # M[p, i, k] ; step: M'[i, j] = max_k(M[i, k] + A2[tl][j, k])
        Mt = vs_p.tile([128, K * K], F32, tag="Mt")
        nc.vector.tensor_copy(out=Mt[:].rearrange("p (k j) -> p k j", k=K),
                              in_=At_r[:, 0, :, :].rearrange("p j k -> p k j"))
        for tl in range(1, NSEG):
            sb = vs_p.tile([128, K * K * K], F32, tag="sb")
            sb_r = sb[:].rearrange("p (i j k) -> p i j k", i=K, j=K)
            nc.vector.tensor_tensor(
                out=sb_r[:, :, :, :],
                in0=Mt[:].rearrange("p (i k) -> p i k", i=K).unsqueeze(2)
                    .broadcast_to((128, K, K, K)),
                in1=At_r[:, tl, :, :].unsqueeze(1)
                    .broadcast_to((128, K, K, K)),
                op=AL.add)
            Mt = vs_p.tile([128, K * K], F32, tag="Mt")
            nc.vector.tensor_reduce(
                out=Mt[:].rearrange("p (i j) -> p i j", i=K),
                in_=sb_r[:, :, :, :], axis=AX.X, op=AL.max)

        # ---- P5c: boundary pass on 8 partitions: z_s = M_s (x) z_{s-1} ----
        nc.sync.dma_start(
            out=mz_dram.rearrange("(b s ij) -> (b s) ij", b=BL, s=NSEG),
            in_=Mt[:])
        M8 = vbig.tile([128, NSEG * K * K], F32, tag="M8")
        nc.sync.dma_start(out=M8[0:BL, :],
                          in_=mz_dram.rearrange("(b sij) -> b sij", b=BL))
        M8_r = M8[:].rearrange("p (s k j) -> p s k j", s=NSEG, k=K)
        zbuf = vbig.tile([128, NSEG * K], F32, tag="zbuf")
        zbuf_r = zbuf[:].rearrange("p (s k) -> p s k", s=NSEG)
        nc.vector.memset(zbuf[0:BL, :], 0.0)
        for s in range(NSEG - 1):
            s3 = vit_p.tile([128, K * K], F32, tag="s3")
            s3_r = s3[:].rearrange("p (j i) -> p j i", j=K)
            nc.vector.tensor_tensor(
                out=s3_r[0:BL, :, :],
                in0=zbuf_r[0:BL, s, :].unsqueeze(1).broadcast_to((BL, K, K)),
                in1=M8_r[0:BL, s, :, :].rearrange("p k j -> p j k"),
                op=AL.add)
            nc.vector.tensor_reduce(
                out=zbuf_r[0:BL, s + 1, :], in_=s3_r[0:BL, :, :],
                axis=AX.X, op=AL.max)

        # relayout z starts -> [(b,seg) partitions, k]
        nc.sync.dma_start(
            out=zb_dram.rearrange("(b sk) -> b sk", b=BL),
            in_=zbuf[0:BL, :])
        zstart = vs_p.tile([128, K], F32, tag="zstart")
        nc.sync.dma_start(out=zstart[:],
                          in_=zb_dram.rearrange("(p k) -> p k", p=128))

        # ---- P5d: phase 2 — within-segment forward scan, all segs parallel ----
        pre_seg = state.tile([128, NSEG * K], F32)
        pre_r = pre_seg[:].rearrange("p (tl k) -> p tl k", tl=NSEG)
        prev_ap = zstart[:, :]
        for tl in range(NSEG):
            s4 = vit_p.tile([128, K * K], F32, tag="s4")
            s4_r = s4[:].rearrange("p (j k) -> p j k", j=K)
            nc.vector.tensor_tensor(
                out=s4_r[:, :, :],
                in0=prev_ap.unsqueeze(1).broadcast_to((128, K, K)),
                in1=At_r[:, tl, :, :],
                op=AL.add)
            nc.vector.tensor_reduce(
                out=pre_r[:, tl, :], in_=s4_r[:, :, :], axis=AX.X, op=AL.max)
            prev_ap = pre_r[:, tl, :]

        # dump pre -> pre_dram in (b t k) order
        nc.sync.dma_start(
            out=pre_dram.rearrange("(b th s2 tl k) -> (b th s2) (tl k)",
                                   b=BL, th=2, s2=8, tl=16),
            in_=pre_seg[:])

        # ---- P5e: end tag via indirect gather of pre[b, n_b-1, :] ----
        peG = vit_p.tile([128, K], F32, tag="peG")
        nc.gpsimd.indirect_dma_start(
            out=peG[0:BL, :], out_offset=None,
            in_=pre_dram.rearrange("(r k) -> r k", k=K),
            in_offset=IndirectOffsetOnAxis(ap=endrow[:, 0:1], axis=0))
        mvE = vit_p.tile([128, 1], F32, tag="mvE")
        nc.vector.tensor_reduce(out=mvE[0:BL, :], in_=peG[0:BL, :], axis=AX.X, op=AL.max)
        eqE = vit_p.tile([128, K], F32, tag="eqE")
        nc.vector.tensor_tensor(out=eqE[0:BL, :], in0=peG[0:BL, :],
                                in1=mvE[0:BL, :].broadcast_to((BL, K)), op=AL.is_equal)
        nc.vector.tensor_mul(out=eqE[0:BL, :], in0=eqE[0:BL, :], in1=iotaD[0:BL, :])
        endt8 = vit_p.tile([128, 1], F32, tag="endt8")
        nc.vector.tensor_reduce(out=endt8[0:BL, :], in_=eqE[0:BL, :], axis=AX.X, op=AL.max)
        nc.vector.tensor_scalar(out=endt8[0:BL, :], in0=endt8[0:BL, :],
                                scalar1=-1.0, scalar2=9.0, op0=AL.mult, op1=AL.add)

        # ---- P6: batched backpointer extraction (from pre_dram, (b t k)) ----
        preT = vbig.tile([128, 2 * BL * K], F32, tag="preT")
        nc.sync.dma_start(
            out=preT[:].rearrange("p (bh k) -> p bh k", bh=2 * BL),
            in_=pre_dram.rearrange("(b h p k) -> p (b h) k", b=BL, h=2, p=128))

        HB = 2 * BL
        preT_hb = preT[:].rearrange("p (hb k) -> p hb k", k=K)
        sX = vbig.tile([128, 2 * BL * K * K], F32, tag="sX")
        sX_r = sX[:].rearrange("p (hb c q) -> p hb c q", hb=HB, c=K)
        nc.vector.tensor_tensor(
            out=sX_r[:, :, :, :],
            in0=preT_hb.unsqueeze(2).broadcast_to((128, HB, K, K)),
            in1=transR[:, :].rearrange("p (c q) -> p c q", c=K).unsqueeze(1)
                .broadcast_to((128, HB, K, K)),
            op=AL.add)
        mX = vbig.tile([128, 2 * BL * K], F32, tag="mX")
        mX_r = mX[:].rearrange("p (hb c) -> p hb c", hb=HB)
        nc.vector.tensor_reduce(out=mX_r[:, :, :], in_=sX_r[:, :, :, :],
                                axis=AX.X, op=AL.max)
        eq = vbig.tile([128, 2 * BL * K * K], F32, tag="eq")
        eq_r = eq[:].rearrange("p (hb c q) -> p hb c q", hb=HB, c=K)
        nc.vector.tensor_tensor(
            out=eq_r[:, :, :, :], in0=sX_r[:, :, :, :],
            in1=mX_r[:, :, :].unsqueeze(3).broadcast_to((128, HB, K, K)),
            op=AL.is_equal)
        nc.vector.tensor_tensor(
            out=eq_r[:, :, :, :], in0=eq_r[:, :, :, :],
            in1=iotaD[:, :].unsqueeze(1).unsqueeze(1).broadcast_to((128, HB, K, K)),
            op=AL.mult)
        bq = vbig.tile([128, 2 * BL * K], F32, tag="bq")
        bq_r = bq[:].rearrange("p (hb c) -> p hb c", hb=HB)
        nc.vector.tensor_reduce(out=bq_r[:, :, :], in_=eq_r[:, :, :, :],
                                axis=AX.X, op=AL.max)
        # bp = 9 - bq ; then pad override: bp*valid + iota_cur*(1-valid)
        nc.vector.tensor_scalar(out=bq[:], in0=bq[:], scalar1=-1.0, scalar2=9.0,
                                op0=AL.mult, op1=AL.add)
        nc.vector.tensor_tensor(
            out=bq_r[:, :, :], in0=bq_r[:, :, :],
            in1=validT[:, :].unsqueeze(2).broadcast_to((128, HB, K)),
            op=AL.mult)
        nc.vector.tensor_tensor(
            out=bq_r[:, :, :], in0=bq_r[:, :, :],
            in1=ivT[:, :].rearrange("p (hb k) -> p hb k", k=K),
            op=AL.add)
        # bp_dram slot t holds the map f_{t+1} (transition into t+1)
        nc.sync.dma_start(
            out=bp_dram[:, :].rearrange("b (h p k) -> p (b h) k", h=2, p=128),
            in_=bq[:].rearrange("p (bh k) -> p bh k", bh=2 * BL))

        # ---- P7: segmented backtrace ----
        # ftile[p=(b,seg), tl, j] = f at u = seg*16+tl, for tl = 1..15
        # (slot u lives at bp_dram position u-1 = seg*16 + (tl-1))
        ftile = state.tile([128, NSEG * K], F32)
        nc.sync.dma_start(
            out=ftile[:, K:],
            in_=bp_dram[:, :].rearrange(
                "b (th s2 tl k) -> (b th s2) (tl k)", th=2, s2=8, tl=16)[:, 0:150])
        ft_r = ftile[:].rearrange("p (tl j) -> p tl j", tl=NSEG)
        # fend8[b, s-1, j] = f_{16s} (= bp_dram position 16s-1), s = 1..15
        fend8 = vbig.tile([128, 15 * K], F32, tag="fend8")
        nc.sync.dma_start(
            out=fend8[0:BL, :].rearrange("p (s k) -> p s k", s=15),
            in_=bp_dram[:, 150:2550].rearrange("b (s gk) -> b s gk",
                                               s=15)[:, :, 0:K])
        fend8_r = fend8[:].rearrange("p (s k) -> p s k", s=15)

        # phase 1: compose G'_s = f_{16s+1} o ... o f_{16s+15}
        Ct = vs_p.tile([128, K], F32, tag="Ct")
        nc.vector.tensor_copy(out=Ct[:], in_=ft_r[:, NSEG - 1, :])
        for tl in range(NSEG - 2, 0, -1):
            ohB = vit_p.tile([128, K * K], F32, tag="ohB")
            ohB_r = ohB[:].rearrange("p (i j) -> p i j", i=K)
            nc.vector.tensor_tensor(
                out=ohB_r[:, :, :],
                in0=Ct[:].unsqueeze(2).broadcast_to((128, K, K)),
                in1=iotaK[:, :].unsqueeze(1).broadcast_to((128, K, K)),
                op=AL.is_equal)
            nc.vector.tensor_tensor(
                out=ohB_r[:, :, :], in0=ohB_r[:, :, :],
                in1=ft_r[:, tl, :].unsqueeze(1).broadcast_to((128, K, K)),
                op=AL.mult)
            Ct = vs_p.tile([128, K], F32, tag="Ct")
            nc.vector.tensor_reduce(out=Ct[:], in_=ohB_r[:, :, :],
                                    axis=AX.X, op=AL.max)

        # relayout G -> [8, (s, i)]
        nc.sync.dma_start(
            out=g_dram.rearrange("(b s i) -> (b s) i", b=BL, s=NSEG), in_=Ct[:])
        G8 = vbig.tile([128, NSEG * K], F32, tag="G8")
        nc.sync.dma_start(out=G8[0:BL, :],
                          in_=g_dram.rearrange("(b si) -> b si", b=BL))
        G8_r = G8[:].rearrange("p (s i) -> p s i", s=NSEG)

        # boundary pass: r_{s-1} = f_{16s}(G'_s(r_s)), r_15 = end tag
        rbuf = vbig.tile([128, NSEG], F32, tag="rbuf")
        nc.vector.tensor_copy(out=rbuf[0:BL, NSEG - 1:NSEG], in_=endt8[0:BL, :])
        for s in range(NSEG - 1, 0, -1):
            oh8 = vit_p.tile([128, K], F32, tag="oh8")
            nc.vector.tensor_tensor(
                out=oh8[0:BL, :], in0=iotaK[0:BL, :],
                in1=rbuf[0:BL, s:s + 1].broadcast_to((BL, K)), op=AL.is_equal)
            nc.vector.tensor_mul(out=oh8[0:BL, :], in0=oh8[0:BL, :],
                                 in1=G8_r[0:BL, s, :])
            tG = vit_p.tile([128, 1], F32, tag="tG")
            nc.vector.tensor_reduce(out=tG[0:BL, :], in_=oh8[0:BL, :],
                                    axis=AX.X, op=AL.max)
            oh9 = vit_p.tile([128, K], F32, tag="oh9")
            nc.vector.tensor_tensor(
                out=oh9[0:BL, :], in0=iotaK[0:BL, :],
                in1=tG[0:BL, :].broadcast_to((BL, K)), op=AL.is_equal)
            nc.vector.tensor_mul(out=oh9[0:BL, :], in0=oh9[0:BL, :],
                                 in1=fend8_r[0:BL, s - 1, :])
            nc.vector.tensor_reduce(out=rbuf[0:BL, s - 1:s], in_=oh9[0:BL, :],
                                    axis=AX.X, op=AL.max)

        # relayout r -> [(b,seg) partitions, 1]
        nc.sync.dma_start(out=r_dram.rearrange("(b s) -> b s", b=BL),
                          in_=rbuf[0:BL, :])
        rstart = vs_p.tile([128, 1], F32, tag="rstart")
        nc.sync.dma_start(out=rstart[:],
                          in_=r_dram.rearrange("(p one) -> p one", one=1))

        # phase 2: walk back within each segment, all segs parallel
        bestseg = state.tile([128, NSEG], F32)
        nc.vector.tensor_copy(out=bestseg[:, NSEG - 1:NSEG], in_=rstart[:])
        for tl in range(NSEG - 1, 0, -1):
            oh2 = vit_p.tile([128, K], F32, tag="oh2")
            nc.vector.tensor_tensor(
                out=oh2[:, :], in0=iotaK[:, :],
                in1=bestseg[:, tl:tl + 1].broadcast_to((128, K)), op=AL.is_equal)
            nc.vector.tensor_mul(out=oh2[:, :], in0=oh2[:, :], in1=ft_r[:, tl, :])
            nc.vector.tensor_reduce(out=bestseg[:, tl - 1:tl], in_=oh2[:, :],
                                    axis=AX.X, op=AL.max)

        # bestseg[p=(b,seg), tl] -> [8, 256], mask, out
        nc.sync.dma_start(
            out=b_dram.rearrange("(b th s2 tl) -> (b th s2) tl",
                                 b=BL, th=2, s2=8),
            in_=bestseg[:])
        best8 = state.tile([128, T], F32)
        nc.sync.dma_start(out=best8[0:BL, :],
                          in_=b_dram.rearrange("(b t) -> b t", b=BL))
        bestM = state.tile([128, T], F32)
        nc.vector.tensor_mul(out=bestM[0:BL, :], in0=best8[0:BL, :],
                             in1=maskBT[0:BL, :])
        nc.sync.dma_start(out=out_d, in_=bestM[0:BL, :])
        ctx.close()

    nc.compile()
    return nc


_NC_CACHE = None


def _get_nc():
    global _NC_CACHE
    if _NC_CACHE is None:
        _NC_CACHE = _build()
    return _NC_CACHE


TRACE = False
LAST_EXEC_NS = None


def kernel(**inputs) -> np.ndarray:
    global LAST_EXEC_NS
    nc = _get_nc()
    in_maps = [_prep_core(inputs, c) for c in range(NC_)]
    res = run_bass_kernel_spmd(nc, in_maps, list(range(NC_)), trace=TRACE)
    LAST_EXEC_NS = res.exec_time_ns
    out = np.concatenate([res.results[c]['out'] for c in range(NC_)], axis=0)
    return out.astype(np.float32)


if __name__ == '__main__':
    _build()
    print("build ok")

